# revision 1
# baseline (speedup 1.0000x reference)
"""Causal single-head attention (B=4, S=2048, D=1024) on 8 TRN2 NeuronCores.

Sharding: core c -> (batch b = c//2, half h = c%2). Every core runs the SAME
program: 8 query tiles of 128 rows whose padded causal key-lengths are
L_s = 256*(s+1) for s=0..7.  Core (b, h) takes global query rows
[256*s + 128*h, 256*s + 128*h + 128) of batch b for slot s.  The last 256 key
columns of each score tile get a data-driven causal mask (depends only on h).

All matmuls run in float32r (TF32-like) with fp32 PSUM accumulation.
Phase order K -> V -> Q -> attention keeps K^T, V and Q^T simultaneously
resident without spilling (Q^T is built last, with W_q streamed per output
chunk).
"""

import numpy as np

import concourse.bacc as bacc
import concourse.mybir as mybir
import concourse.tile as tile
from concourse import bass_utils

B, S, D = 4, 2048, 1024
P = 128
DC = D // P          # 8 contraction chunks
EC = D // P          # 8 output-dim chunks
NSLOT = 8            # q tiles per core
NQ = NSLOT * P       # 1024 q rows per core
SCALE = 1.0 / float(np.sqrt(np.float32(S)))
NEG = -1.0e9

F32 = mybir.dt.float32
F32R = mybir.dt.float32r
BF16 = mybir.dt.bfloat16


def build_attention_nc():
    nc = bacc.Bacc("TRN2", target_bir_lowering=False)

    xq = nc.dram_tensor("xq", [NQ, D], F32R, kind="ExternalInput")
    xk = nc.dram_tensor("xk", [S, D], F32R, kind="ExternalInput")
    xv = nc.dram_tensor("xv", [S, D], F32R, kind="ExternalInput")
    wq = nc.dram_tensor("wq", [EC, P, DC, P], F32R, kind="ExternalInput")
    wk = nc.dram_tensor("wk", [EC, P, DC, P], F32R, kind="ExternalInput")
    wv = nc.dram_tensor("wv", [EC, P, DC, P], F32R, kind="ExternalInput")
    mask = nc.dram_tensor("mask", [P, 256], BF16, kind="ExternalInput")
    ident_in = nc.dram_tensor("ident", [P, P], F32R, kind="ExternalInput")
    out = nc.dram_tensor("out", [NQ, D], F32, kind="ExternalOutput")



    with tile.TileContext(nc) as tc:
        with (
            tc.tile_pool(name="res", bufs=1) as res,
            tc.tile_pool(name="xrowq", bufs=3) as xrowqp,
        ):
            kt_sb = res.tile([P, EC, S], F32R)      # K^T  [e, keys]
            v_sb = res.tile([P, S // P, D], F32R)   # V    [keys, e]
            ident = res.tile([P, P], F32R)
            nc.scalar.dma_start(ident, ident_in[:, :])

            # ================= K / V projection phases =================
            with (
                tc.tile_pool(name="wpool", bufs=1) as wpool,
                tc.tile_pool(name="xrow", bufs=2) as xrowp,
                tc.tile_pool(name="xt", bufs=2) as xtp,
                tc.tile_pool(name="pp", bufs=5, space="PSUM") as pp,
                tc.tile_pool(name="pt", bufs=3, space="PSUM") as pt,
            ):

                def load_xT(x_dram, r0, width, split_first=False):
                    """Transpose `width` rows of X starting at r0 into [P, DC, width]."""
                    blk = xtp.tile([P, DC, width], F32R, tag=f"xtblk{width}")
                    for st in range(width // P):
                        xrow = xrowp.tile([P, D], F32R, tag="xrow")
                        rlo = r0 + st * P
                        if split_first:
                            nc.sync.dma_start(
                                xrow[:, 0:512], x_dram[rlo : rlo + P, 0:512]
                            )
                            nc.sync.dma_start(
                                xrow[:, 512:D], x_dram[rlo : rlo + P, 512:D]
                            )
                        else:
                            nc.sync.dma_start(xrow, x_dram[rlo : rlo + P, :])
                        for dc4 in range(2):
                            ptile = pt.tile([P, 4, P], F32R, tag="ptr")
                            for i in range(4):
                                dc = dc4 * 4 + i
                                nc.tensor.transpose(
                                    ptile[:, i, :],
                                    xrow[:, dc * P : (dc + 1) * P],
                                    ident,
                                )
                            nc.vector.tensor_copy(
                                blk[:, dc4 * 4 : dc4 * 4 + 4, st * P : (st + 1) * P],
                                ptile,
                            )
                    return blk

                w_sb = wpool.tile([P, EC, DC, P], F32R, tag="w")

                def load_w(w_t):
                    """W load chunked by output columns, on the ACT HWDGE queue.

                    Reuses the same tile across phases: per-ec WAR deps let the
                    next phase's chunks stream in as the old ones retire.
                    w_sb layout: [p, ec, dc, q] with e = ec*128 + q; each per-ec
                    chunk is a contiguous 4KB run per partition on both sides."""
                    for ec in range(EC):
                        nc.scalar.dma_start(w_sb[:, ec], w_t[ec])
                    return w_sb

                # ---- phase K: K^T resident ----
                blk = load_xT(xk, 0, 512, split_first=True)
                for ec in range(2):
                    nc.scalar.dma_start(w_sb[:, ec], wk[ec])
                for kb in range(S // 512):
                    if kb > 0:
                        blk = load_xT(xk, kb * 512, 512)
                    for ec in range(EC):
                        ps = pp.tile([P, 512], F32, tag="pmm")
                        for half in range(2):
                            for dc in range(DC):
                                nc.tensor.matmul(
                                    ps[:, half * 256 : (half + 1) * 256],
                                    w_sb[:, ec, dc, :],
                                    blk[:, dc, half * 256 : (half + 1) * 256],
                                    start=(dc == 0),
                                    stop=(dc == DC - 1),
                                )
                        nc.scalar.copy(
                            kt_sb[:, ec, kb * 512 : (kb + 1) * 512], ps
                        )
                        if kb == 0 and ec < EC - 2:
                            nc.scalar.dma_start(w_sb[:, ec + 2], wk[ec + 2])

                # ---- phase V: V resident ----
                load_w(wv)
                for kb in range(S // 512):
                    blk = load_xT(xv, kb * 512, 512)
                    for st in range(4):
                        kc = kb * 4 + st
                        for eh in range(2):
                            ps = pp.tile([P, 512], F32, tag="pmm")
                            for dc in range(DC):
                                nc.tensor.matmul(
                                    ps,
                                    blk[:, dc, st * P : (st + 1) * P],
                                    w_sb[:, eh * 4 : (eh + 1) * 4, dc, :],
                                    start=(dc == 0),
                                    stop=(dc == DC - 1),
                                )
                            nc.scalar.copy(
                                v_sb[:, kc, eh * 512 : (eh + 1) * 512], ps
                            )

            # ========== Q projection phase (Q^T resident, W streamed) ==========
            with tc.tile_pool(name="qtp", bufs=1) as qtp:
              qt_sb = qtp.tile([P, EC, NQ], F32R)     # Q^T  [e, q]
              with (
                tc.tile_pool(name="xtq", bufs=1) as xtqp,
                tc.tile_pool(name="wqp", bufs=1) as wqp,
                tc.tile_pool(name="ppq", bufs=4, space="PSUM") as ppq,
                tc.tile_pool(name="ptq", bufs=4, space="PSUM") as ptq,
              ):
                w_sb = wqp.tile([P, EC, DC, P], F32R, tag="wq")
                for bi in range(4):
                    blk = xtqp.tile([P, DC, 256], F32R, tag="xtq")
                    for st in range(2):
                        r0 = bi * 256 + st * P
                        if bi == 0:
                            for ec4 in range(4):
                                ec = st * 4 + ec4
                                nc.scalar.dma_start(w_sb[:, ec], wq[ec])
                        for dc4 in range(2):
                            xrow = xrowqp.tile([P, 512], F32R, tag="xrowq")
                            nc.sync.dma_start(
                                xrow, xq[r0 : r0 + P, dc4 * 512 : (dc4 + 1) * 512]
                            )
                            ptile = ptq.tile([P, 4, P], F32R, tag="ptrq")
                            for i in range(4):
                                nc.tensor.transpose(
                                    ptile[:, i, :],
                                    xrow[:, i * P : (i + 1) * P],
                                    ident,
                                )
                            nc.vector.tensor_copy(
                                blk[:, dc4 * 4 : dc4 * 4 + 4, st * P : (st + 1) * P],
                                ptile,
                            )
                    for ec in range(EC):
                        ps = ppq.tile([P, 256], F32, tag="pmq")
                        for dc in range(DC):
                            nc.tensor.matmul(
                                ps,
                                w_sb[:, ec, dc, :],
                                blk[:, dc, :],
                                start=(dc == 0),
                                stop=(dc == DC - 1),
                            )
                        nc.scalar.copy(
                            qt_sb[:, ec, bi * 256 : (bi + 1) * 256], ps
                        )

              # ================= attention phase =================
              with (
                  tc.tile_pool(name="attn", bufs=2) as attnp,
                  tc.tile_pool(name="psc", bufs=4, space="PSUM") as psc,
                  tc.tile_pool(name="pta", bufs=2, space="PSUM") as pta,
                  tc.tile_pool(name="po", bufs=2, space="PSUM") as po,
              ):
                  mask_sb = attnp.tile([P, 256], BF16, tag="mask")
                  nc.sync.dma_start(mask_sb, mask[:, :])
                  for s in range(NSLOT):
                      L = 256 * (s + 1)
                      nj = (L + 511) // 512
                      nt = L // P

                      attn_sb = attnp.tile([P, S], F32R, tag="attn")
                      acc = attnp.tile([P, 4], F32, tag="acc")
                      ps_list = []
                      for j in range(nj):
                          w_j = min(512, L - j * 512)
                          ps = psc.tile([P, 512], F32, tag="ps_sc")
                          ps_list.append((ps, w_j))
                      for ec in range(EC):
                          for j, (ps, w_j) in enumerate(ps_list):
                              nc.tensor.matmul(
                                  ps[:, :w_j],
                                  qt_sb[:, ec, s * P : (s + 1) * P],
                                  kt_sb[:, ec, j * 512 : j * 512 + w_j],
                                  start=(ec == 0),
                                  stop=(ec == EC - 1),
                              )
                      # causal mask on the last 256 key columns
                      ps_last, w_last = ps_list[-1]
                      off = w_last - 256
                      nc.vector.tensor_add(
                          out=ps_last[:, off : off + 256],
                          in0=ps_last[:, off : off + 256],
                          in1=mask_sb,
                      )
                      # exp + per-chunk row sums
                      for j, (ps, w_j) in enumerate(ps_list):
                          nc.scalar.activation(
                              out=attn_sb[:, j * 512 : j * 512 + w_j],
                              in_=ps[:, :w_j],
                              func=mybir.ActivationFunctionType.Exp,
                              scale=SCALE,
                              accum_out=acc[:, j : j + 1],
                          )
                      total = attnp.tile([P, 1], F32, tag="total")
                      nc.vector.tensor_reduce(
                          total,
                          acc[:, :nj],
                          axis=mybir.AxisListType.X,
                          op=mybir.AluOpType.add,
                      )
                      rec = attnp.tile([P, 1], F32, tag="rec")
                      nc.vector.reciprocal(rec, total)

                      # transpose attn -> attnT [keys, q]
                      attnT = attnp.tile([P, S // P, P], F32R, tag="attnT")
                      for t4 in range((nt + 3) // 4):
                          cnt = min(4, nt - t4 * 4)
                          ptile = pta.tile([P, 4, P], F32R, tag="pta")
                          for i in range(cnt):
                              t = t4 * 4 + i
                              nc.tensor.transpose(
                                  ptile[:, i, :], attn_sb[:, t * P : (t + 1) * P], ident
                              )
                          nc.vector.tensor_copy(
                              attnT[:, t4 * 4 : t4 * 4 + cnt, :], ptile[:, :cnt, :]
                          )

                      # attn @ V, normalized on copy-out
                      out_sb = attnp.tile([P, D], F32, tag="out", bufs=1)
                      for eh in range(2):
                          ps_o = po.tile([P, 512], F32, tag="ps_o")
                          for t in range(nt):
                              nc.tensor.matmul(
                                  ps_o,
                                  attnT[:, t, :],
                                  v_sb[:, t, eh * 512 : (eh + 1) * 512],
                                  start=(t == 0),
                                  stop=(t == nt - 1),
                              )
                          nc.scalar.activation(
                              out=out_sb[:, eh * 512 : (eh + 1) * 512],
                              in_=ps_o,
                              func=mybir.ActivationFunctionType.Copy,
                              scale=rec,
                          )
                      nc.sync.dma_start(out[s * P : (s + 1) * P, :], out_sb)

    nc.compile()
    return nc


_NC_CACHE = None


def _get_nc():
    global _NC_CACHE
    if _NC_CACHE is None:
        _NC_CACHE = build_attention_nc()
    return _NC_CACHE


def _make_mask(h: int) -> np.ndarray:
    import ml_dtypes

    i = np.arange(P)[:, None]
    j = np.arange(256)[None, :]
    allowed = j <= (i + 128 * h)
    return np.where(allowed, 0.0, NEG).astype(ml_dtypes.bfloat16)


def kernel(
    inputs_for_keys,
    inputs_for_values,
    inputs_for_queries,
    weight_K,
    weight_V,
    weight_Q,
    trace=False,
):
    xk_full = np.ascontiguousarray(np.asarray(inputs_for_keys, dtype=np.float32))
    xv_full = np.ascontiguousarray(np.asarray(inputs_for_values, dtype=np.float32))
    xq_full = np.ascontiguousarray(np.asarray(inputs_for_queries, dtype=np.float32))
    def _reorder_w(w):
        w = np.asarray(w, dtype=np.float32).reshape(DC, P, EC, P)
        return np.ascontiguousarray(w.transpose(2, 1, 0, 3))

    w_k = _reorder_w(weight_K)
    w_v = _reorder_w(weight_V)
    w_q = _reorder_w(weight_Q)

    masks = [_make_mask(0), _make_mask(1)]
    ident_np = np.eye(P, dtype=np.float32)
    in_maps = []
    for c in range(2 * B):
        b, h = c // 2, c % 2
        rows = np.concatenate(
            [
                xq_full[b, 256 * s + 128 * h : 256 * s + 128 * h + P, :]
                for s in range(NSLOT)
            ],
            axis=0,
        )
        in_maps.append(
            {
                "xq": np.ascontiguousarray(rows),
                "xk": xk_full[b],
                "xv": xv_full[b],
                "wq": w_q,
                "wk": w_k,
                "wv": w_v,
                "mask": masks[h],
                "ident": ident_np,
            }
        )

    nc = _get_nc()
    res = bass_utils.run_bass_kernel_spmd(
        nc, in_maps, core_ids=list(range(2 * B)), trace=trace
    )

    out = np.empty((B, S, D), dtype=np.float32)
    for c in range(2 * B):
        b, h = c // 2, c % 2
        o = res.results[c]["out"]
        for s in range(NSLOT):
            out[b, 256 * s + 128 * h : 256 * s + 128 * h + P, :] = o[
                s * P : (s + 1) * P, :
            ]

    if trace:
        return out, res
    return out



# revision 6
# speedup vs baseline: 1.5980x; 1.5980x over previous
"""Causal single-head attention (B=4, S=2048, D=1024) on 8 TRN2 NeuronCores.

Sharding: core c -> (batch b = c//2, half h = c%2). Every core runs the SAME
program: its 1024 query rows are 8 slots of 128 rows; slot s holds global
rows [256*s + 128*h, 256*s + 128*h + 128) of batch b, whose padded causal
key-length is 256*(s+1).

All matmuls run as fp8(e4m3) DoubleRow (2 contraction tiles per instruction,
0.5 cycles/row) with 3-term hi/lo error compensation:
    x @ w ~= xh@wh + xh@wl + xl@wh        (drop xl@wl, ~0.1% error)
X^T and W (pre-scaled by 32) are split hi/lo on the host and shipped fp8, so
the kernel needs no PE transposes for inputs. Scores are computed transposed
(S^T[k, q] with keys on partitions) so the attention weights can be consumed
directly as DoubleRow stationaries by attn @ V -- no attention transposes
either. exp() output is split hi/lo on device (ACT copy + DVE subtract).
The softmax denominator comes from an extra ones-column DoubleRow matmul and
cancels the 32x V scale exactly. The causal mask is one 128x128 f32 add per
key-chunk (two host mask tiles, selected by parity).
"""

import numpy as np

import concourse.bacc as bacc
import concourse.mybir as mybir
import concourse.tile as tile
from concourse import bass_utils

B, S, D = 4, 2048, 1024
P = 128
DCP = 4              # pairs of 128-deep contraction tiles (d dim)
ECP = 4              # pairs of 128-wide e tiles
NSLOT = 8            # q tiles per core
NQ = NSLOT * P       # 1024 q rows per core
NKC = S // P         # 16 key chunks
WSCALE = 32.0        # host pre-scale on all three weights
SCALE_EFF = 1.0 / (WSCALE * WSCALE * float(np.sqrt(np.float32(S))))
NEG = -1.0e9

F32 = mybir.dt.float32
FP8 = mybir.dt.float8e4
DR = mybir.MatmulPerfMode.DoubleRow


def build_attention_nc():
    nc = bacc.Bacc("TRN2", target_bir_lowering=False)

    xk_h = nc.dram_tensor("xk_h", [P, DCP, 2, S], FP8, kind="ExternalInput")
    xk_l = nc.dram_tensor("xk_l", [P, DCP, 2, S], FP8, kind="ExternalInput")
    xv_h = nc.dram_tensor("xv_h", [P, DCP, 2, S], FP8, kind="ExternalInput")
    xv_l = nc.dram_tensor("xv_l", [P, DCP, 2, S], FP8, kind="ExternalInput")
    xq_h = nc.dram_tensor("xq_h", [P, DCP, 2, NQ], FP8, kind="ExternalInput")
    xq_l = nc.dram_tensor("xq_l", [P, DCP, 2, NQ], FP8, kind="ExternalInput")
    w_in = {}
    for t in ("k", "v", "q"):
        for c in ("h", "l"):
            w_in[t, c] = nc.dram_tensor(
                f"w{t}_{c}", [P, DCP, 2, D], FP8, kind="ExternalInput"
            )
    mask_a = nc.dram_tensor("mask_a", [P, P], F32, kind="ExternalInput")
    mask_b = nc.dram_tensor("mask_b", [P, P], F32, kind="ExternalInput")
    ones_in = nc.dram_tensor("ones32", [P, 2, 1], FP8, kind="ExternalInput")
    out = nc.dram_tensor("out", [NQ, D], F32, kind="ExternalOutput")

    with tile.TileContext(nc) as tc:
        with (
            tc.tile_pool(name="res", bufs=1) as res,
            tc.tile_pool(name="wp", bufs=2) as wp,
            tc.tile_pool(name="xs", bufs=2) as xs,
            tc.tile_pool(name="tmpp", bufs=3) as tmpp,
            tc.tile_pool(name="outp", bufs=2) as outp,
            tc.tile_pool(name="recp", bufs=2) as recp,
        ):
            kt = {c: res.tile([P, ECP, 2, S], FP8, tag=f"kt{c}", name=f"kt{c}") for c in "hl"}
            vv = {c: res.tile([P, NKC, D], FP8, tag=f"v{c}", name=f"v{c}") for c in "hl"}
            qt = {c: res.tile([P, ECP, 2, NQ], FP8, tag=f"qt{c}", name=f"qt{c}") for c in "hl"}
            at = {c: res.tile([P, NKC, NQ], FP8, tag=f"at{c}", name=f"at{c}") for c in "hl"}
            ma_sb = res.tile([P, P], F32, tag="maska")
            mb_sb = res.tile([P, P], F32, tag="maskb")
            ones_sb = res.tile([P, 2, 1], FP8, tag="ones")
            nc.scalar.dma_start(ma_sb, mask_a[:, :])
            nc.scalar.dma_start(mb_sb, mask_b[:, :])
            nc.scalar.dma_start(ones_sb, ones_in[:, :, :])

            def proj_psum(ps, stats, movs, stat_slice, mov_slice):
                """12 DR matmuls: terms (h,h),(h,l),(l,h) x 4 dc pairs."""
                n = 0
                for cs, cm in (("h", "h"), ("h", "l"), ("l", "h")):
                    for dcp in range(DCP):
                        nc.tensor.matmul(
                            ps,
                            stat_slice(stats[cs], dcp),
                            mov_slice(movs[cm], dcp),
                            start=(n == 0),
                            stop=(n == 11),
                            perf_mode=DR,
                        )
                        n += 1

            def store_hilo(ps, hi_slice, lo_slice):
                nc.scalar.copy(hi_slice, ps)
                nc.vector.tensor_tensor(
                    out=lo_slice, in0=ps, in1=hi_slice,
                    op=mybir.AluOpType.subtract,
                )

            # ================= K / V / Q projection =================
            with tc.tile_pool(name="pp", bufs=5, space="PSUM") as pp:
                wk = {c: wp.tile([P, DCP, 2, D], FP8, tag=f"w{c}", name=f"wk{c}") for c in "hl"}
                for c in "hl":
                    nc.scalar.dma_start(wk[c], w_in["k", c][:, :, :, :])

                # ---- K: kt[e, keys] resident ----
                for kb in range(4):
                    xk = {c: xs.tile([P, DCP, 2, 512], FP8, tag="xs", bufs=4, name=f"xk{c}") for c in "hl"}
                    for c in "hl":
                        src = xk_h if c == "h" else xk_l
                        nc.sync.dma_start(xk[c], src[:, :, :, kb * 512 : (kb + 1) * 512])
                    for ec in range(8):
                        ps = pp.tile([P, 512], F32, tag="ps")
                        proj_psum(
                            ps, wk, xk,
                            lambda w, dcp, ec=ec: w[:, dcp, :, ec * P : (ec + 1) * P],
                            lambda x, dcp: x[:, dcp, :, :],
                        )
                        store_hilo(
                            ps,
                            kt["h"][:, ec // 2, ec % 2, kb * 512 : (kb + 1) * 512],
                            kt["l"][:, ec // 2, ec % 2, kb * 512 : (kb + 1) * 512],
                        )

                # ---- V: v[keys, e] resident ----
                wv = {c: wp.tile([P, DCP, 2, D], FP8, tag=f"w{c}", name=f"wv{c}") for c in "hl"}
                for c in "hl":
                    nc.scalar.dma_start(wv[c], w_in["v", c][:, :, :, :])
                for kb in range(4):
                    xv = {c: xs.tile([P, DCP, 2, 512], FP8, tag="xs", bufs=4, name=f"xv{c}") for c in "hl"}
                    for c in "hl":
                        src = xv_h if c == "h" else xv_l
                        nc.sync.dma_start(xv[c], src[:, :, :, kb * 512 : (kb + 1) * 512])
                    for kti in range(4):
                        ktg = kb * 4 + kti
                        for eh in range(2):
                            ps = pp.tile([P, 512], F32, tag="ps")
                            proj_psum(
                                ps, xv, wv,
                                lambda x, dcp, kti=kti: x[:, dcp, :, kti * P : (kti + 1) * P],
                                lambda w, dcp, eh=eh: w[:, dcp, :, eh * 512 : (eh + 1) * 512],
                            )
                            store_hilo(
                                ps,
                                vv["h"][:, ktg, eh * 512 : (eh + 1) * 512],
                                vv["l"][:, ktg, eh * 512 : (eh + 1) * 512],
                            )

                # ---- Q: qt[e, q] resident ----
                wq = {c: wp.tile([P, DCP, 2, D], FP8, tag=f"w{c}", name=f"wq{c}") for c in "hl"}
                for c in "hl":
                    nc.scalar.dma_start(wq[c], w_in["q", c][:, :, :, :])
                xq = {c: xs.tile([P, DCP, 2, NQ], FP8, tag=f"xq{c}", bufs=1, name=f"xq{c}") for c in "hl"}
                nc.sync.dma_start(xq["h"], xq_h[:, :, :, :])
                nc.sync.dma_start(xq["l"], xq_l[:, :, :, :])
                for qb in range(2):
                    for ec in range(8):
                        ps = pp.tile([P, 512], F32, tag="ps")
                        proj_psum(
                            ps, wq, xq,
                            lambda w, dcp, ec=ec: w[:, dcp, :, ec * P : (ec + 1) * P],
                            lambda x, dcp, qb=qb: x[:, dcp, :, qb * 512 : (qb + 1) * 512],
                        )
                        store_hilo(
                            ps,
                            qt["h"][:, ec // 2, ec % 2, qb * 512 : (qb + 1) * 512],
                            qt["l"][:, ec // 2, ec % 2, qb * 512 : (qb + 1) * 512],
                        )

            # ================= attention =================
            with (
                tc.tile_pool(name="psc", bufs=4, space="PSUM") as psc,
                tc.tile_pool(name="pout", bufs=2, space="PSUM") as pout,
                tc.tile_pool(name="pden", bufs=2, space="PSUM") as pden,
            ):

                def score_chunk(t):
                    """S^T[k-chunk t, q] for all slots needing it, + exp hi/lo."""
                    q0 = P * (t // 2)
                    pieces = []
                    pq0 = q0
                    while pq0 < NQ:
                        wp_ = min(512, NQ - pq0)
                        pieces.append((pq0, wp_))
                        pq0 += wp_
                    for pi, (pq0, wp_) in enumerate(pieces):
                        ps = psc.tile([P, 512], F32, tag="sc")
                        n = 0
                        for cs, cm in (("h", "h"), ("h", "l"), ("l", "h")):
                            for ecp in range(ECP):
                                nc.tensor.matmul(
                                    ps[:, :wp_],
                                    kt[cs][:, ecp, :, t * P : (t + 1) * P],
                                    qt[cm][:, ecp, :, pq0 : pq0 + wp_],
                                    start=(n == 0),
                                    stop=(n == 11),
                                    perf_mode=DR,
                                )
                                n += 1
                        if pi == 0:
                            nc.vector.tensor_tensor(
                                out=ps[:, 0:P], in0=ps[:, 0:P],
                                in1=(ma_sb if t % 2 == 0 else mb_sb),
                                op=mybir.AluOpType.add,
                            )
                        tmp = tmpp.tile([P, 512], F32, tag="tmp")
                        nc.scalar.activation(
                            out=tmp[:, :wp_], in_=ps[:, :wp_],
                            func=mybir.ActivationFunctionType.Exp,
                            scale=SCALE_EFF,
                        )
                        store_hilo(
                            tmp[:, :wp_],
                            at["h"][:, t, pq0 : pq0 + wp_],
                            at["l"][:, t, pq0 : pq0 + wp_],
                        )

                def attn_v(s):
                    """out[q-slot s, :] = (a @ 32V) / (a @ 32)."""
                    npair = s + 1
                    ps_den = pden.tile([P, 1], F32, tag="den")
                    n = 0
                    for j in range(npair):
                        for c in "hl":
                            nc.tensor.matmul(
                                ps_den,
                                at[c][:, 2 * j : 2 * j + 2, s * P : (s + 1) * P],
                                ones_sb,
                                start=(n == 0),
                                stop=(n == 2 * npair - 1),
                                perf_mode=DR,
                            )
                            n += 1
                    rec = recp.tile([P, 1], F32, tag="rec")
                    nc.vector.reciprocal(rec, ps_den)
                    out_sb = outp.tile([P, D], F32, tag="outsb")
                    for eh in range(2):
                        ps_o = pout.tile([P, 512], F32, tag="po")
                        n = 0
                        for j in range(npair):
                            for ca, cv in (("h", "h"), ("h", "l"), ("l", "h")):
                                nc.tensor.matmul(
                                    ps_o,
                                    at[ca][:, 2 * j : 2 * j + 2, s * P : (s + 1) * P],
                                    vv[cv][:, 2 * j : 2 * j + 2, eh * 512 : (eh + 1) * 512],
                                    start=(n == 0),
                                    stop=(n == 3 * npair - 1),
                                    perf_mode=DR,
                                )
                                n += 1
                        nc.scalar.activation(
                            out=out_sb[:, eh * 512 : (eh + 1) * 512],
                            in_=ps_o,
                            func=mybir.ActivationFunctionType.Copy,
                            scale=rec,
                        )
                    nc.sync.dma_start(out[s * P : (s + 1) * P, :], out_sb)

                # pipeline: score chunks run one slot ahead of attn_v
                score_chunk(0)
                score_chunk(1)
                for s in range(NSLOT):
                    if s < NSLOT - 1:
                        score_chunk(2 * s + 2)
                        score_chunk(2 * s + 3)
                    attn_v(s)

    nc.compile()
    return nc


_NC_CACHE = None


def _get_nc():
    global _NC_CACHE
    if _NC_CACHE is None:
        _NC_CACHE = build_attention_nc()
    return _NC_CACHE


def _split8(a):
    import ml_dtypes

    f8 = ml_dtypes.float8_e4m3
    h = a.astype(np.float32).astype(f8)
    l = (a.astype(np.float32) - h.astype(np.float32)).astype(f8)
    return h, l


def _to_xt(a):
    """[rows, d] f32 component -> [128, DCP, 2, rows] fp8 (d on partitions)."""
    rows = a.shape[0]
    return np.ascontiguousarray(a.reshape(rows, DCP, 2, P).transpose(3, 1, 2, 0))


def _to_w(a):
    """[d, e] f32 component -> [128, DCP, 2, e] fp8 (d on partitions)."""
    return np.ascontiguousarray(a.reshape(DCP, 2, P, D).transpose(2, 0, 1, 3))


def _make_masks(h):
    i = np.arange(P)[:, None]
    j = np.arange(P)[None, :]
    if h == 0:
        a = np.where(j >= i, 0.0, NEG).astype(np.float32)  # k partition, q free
        b = np.full((P, P), NEG, dtype=np.float32)
    else:
        a = np.zeros((P, P), dtype=np.float32)
        b = np.where(j >= i, 0.0, NEG).astype(np.float32)
    return a, b


def kernel(
    inputs_for_keys,
    inputs_for_values,
    inputs_for_queries,
    weight_K,
    weight_V,
    weight_Q,
    trace=False,
):
    import ml_dtypes

    xk_full = np.asarray(inputs_for_keys, dtype=np.float32)
    xv_full = np.asarray(inputs_for_values, dtype=np.float32)
    xq_full = np.asarray(inputs_for_queries, dtype=np.float32)

    w_split = {}
    for name, w in (("k", weight_K), ("v", weight_V), ("q", weight_Q)):
        wh, wl = _split8(np.asarray(w, dtype=np.float32) * WSCALE)
        w_split[name] = (_to_w(wh), _to_w(wl))

    xk_split = [tuple(_to_xt(c) for c in _split8(xk_full[b])) for b in range(B)]
    xv_split = [tuple(_to_xt(c) for c in _split8(xv_full[b])) for b in range(B)]

    ones32 = np.full((P, 2, 1), WSCALE, dtype=ml_dtypes.float8_e4m3)
    masks = [_make_masks(0), _make_masks(1)]

    in_maps = []
    for c in range(2 * B):
        b, h = c // 2, c % 2
        rows = np.concatenate(
            [
                xq_full[b, 256 * s + P * h : 256 * s + P * h + P, :]
                for s in range(NSLOT)
            ],
            axis=0,
        )
        qh, ql = _split8(rows)
        ma, mb = masks[h]
        in_maps.append(
            {
                "xk_h": xk_split[b][0], "xk_l": xk_split[b][1],
                "xv_h": xv_split[b][0], "xv_l": xv_split[b][1],
                "xq_h": _to_xt(qh), "xq_l": _to_xt(ql),
                "wk_h": w_split["k"][0], "wk_l": w_split["k"][1],
                "wv_h": w_split["v"][0], "wv_l": w_split["v"][1],
                "wq_h": w_split["q"][0], "wq_l": w_split["q"][1],
                "mask_a": ma, "mask_b": mb,
                "ones32": ones32,
            }
        )

    nc = _get_nc()
    res = bass_utils.run_bass_kernel_spmd(
        nc, in_maps, core_ids=list(range(2 * B)), trace=trace
    )

    out = np.empty((B, S, D), dtype=np.float32)
    for c in range(2 * B):
        b, h = c // 2, c % 2
        o = res.results[c]["out"]
        for s in range(NSLOT):
            out[b, 256 * s + P * h : 256 * s + P * h + P, :] = o[
                s * P : (s + 1) * P, :
            ]

    if trace:
        return out, res
    return out


# revision 18
# speedup vs baseline: 1.7834x; 1.1160x over previous
"""Causal single-head attention (B=4, S=2048, D=1024) on 8 TRN2 NeuronCores.

Sharding: core c -> (batch b = c//2, half h = c%2). Every core runs the SAME
program: its 1024 query rows are 8 slots of 128 rows; slot s holds global
rows [256*s + 128*h, 256*s + 128*h + 128) of batch b, whose padded causal
key-length is 256*(s+1).

All matmuls run as fp8(e4m3) DoubleRow (2 contraction tiles per instruction,
0.5 cycles/row) with 3-term hi/lo error compensation:
    x @ w ~= xh@wh + xh@wl + xl@wh        (drop xl@wl, ~0.1% error)
X^T and W (pre-scaled by 32) are split hi/lo on the host and shipped fp8, so
the kernel needs no PE transposes for inputs. Scores are computed transposed
(S^T[k, q] with keys on partitions) so the attention weights can be consumed
directly as DoubleRow stationaries by attn @ V -- no attention transposes
either. exp() output is split hi/lo on device (ACT copy + DVE subtract).
The softmax denominator comes from an extra ones-column DoubleRow matmul and
cancels the 32x V scale exactly. The causal mask is one 128x128 f32 add per
key-chunk (two host mask tiles, selected by parity).
"""

import numpy as np

import concourse.bacc as bacc
import concourse.mybir as mybir
import concourse.tile as tile
from concourse import bass_utils

B, S, D = 4, 2048, 1024
P = 128
DCP = 4              # pairs of 128-deep contraction tiles (d dim)
ECP = 4              # pairs of 128-wide e tiles
NSLOT = 8            # q tiles per core
NQ = NSLOT * P       # 1024 q rows per core
NKC = S // P         # 16 key chunks
WSCALE = 32.0        # host pre-scale on all three weights
SCALE_EFF = 1.0 / (WSCALE * WSCALE * float(np.sqrt(np.float32(S))))
NEG = -1.0e9

F32 = mybir.dt.float32
FP8 = mybir.dt.float8e4
BF16 = mybir.dt.bfloat16
DR = mybir.MatmulPerfMode.DoubleRow


def build_attention_nc():
    nc = bacc.Bacc("TRN2", target_bir_lowering=False)

    xk_h = nc.dram_tensor("xk_h", [P, DCP, 2, S], FP8, kind="ExternalInput")
    xk_l = nc.dram_tensor("xk_l", [P, DCP, 2, S], FP8, kind="ExternalInput")
    xv_h = nc.dram_tensor("xv_h", [P, DCP, 2, S], FP8, kind="ExternalInput")
    xv_l = nc.dram_tensor("xv_l", [P, DCP, 2, S], FP8, kind="ExternalInput")
    xq_h = nc.dram_tensor("xq_h", [P, DCP, 2, NQ], FP8, kind="ExternalInput")
    xq_l = nc.dram_tensor("xq_l", [P, DCP, 2, NQ], FP8, kind="ExternalInput")
    w_in = {}
    for t in ("k", "v", "q"):
        for c in ("h", "l"):
            w_in[t, c] = nc.dram_tensor(
                f"w{t}_{c}", [P, DCP, 2, D], FP8, kind="ExternalInput"
            )
    mask_a = nc.dram_tensor("mask_a", [P, P], F32, kind="ExternalInput")
    mask_b = nc.dram_tensor("mask_b", [P, P], F32, kind="ExternalInput")
    ones_in = nc.dram_tensor("ones32", [P, 2, 1], FP8, kind="ExternalInput")
    out = nc.dram_tensor("out", [NQ, D], BF16, kind="ExternalOutput")

    with tile.TileContext(nc) as tc:
        with (
            tc.tile_pool(name="res", bufs=1) as res,
            tc.tile_pool(name="wp", bufs=2) as wp,
            tc.tile_pool(name="xs", bufs=2) as xs,
            tc.tile_pool(name="tmpp", bufs=3) as tmpp,
            tc.tile_pool(name="outp", bufs=2) as outp,
            tc.tile_pool(name="recp", bufs=2) as recp,
        ):
            # scores run 1-term (hi only): kt/qt keep no lo component
            kt_h = res.tile([P, ECP, 2, S], FP8, tag="kth", name="kt_h")
            vv = {c: res.tile([P, NKC, D], FP8, tag=f"v{c}", name=f"v{c}") for c in "hl"}
            qt_h = res.tile([P, ECP, 2, NQ], FP8, tag="qth", name="qt_h")
            at = {c: res.tile([P, NKC, NQ], FP8, tag=f"at{c}", name=f"at{c}") for c in "hl"}
            ma_sb = res.tile([P, P], F32, tag="maska")
            mb_sb = res.tile([P, P], F32, tag="maskb")
            ones_sb = res.tile([P, 2, 1], FP8, tag="ones")

            def proj_psum(ps, stats, movs, stat_slice, mov_slice):
                """12 DR matmuls: terms (h,h),(h,l),(l,h) x 4 dc pairs."""
                n = 0
                for cs, cm in (("h", "h"), ("h", "l"), ("l", "h")):
                    for dcp in range(DCP):
                        nc.tensor.matmul(
                            ps,
                            stat_slice(stats[cs], dcp),
                            mov_slice(movs[cm], dcp),
                            start=(n == 0),
                            stop=(n == 11),
                            perf_mode=DR,
                        )
                        n += 1

            def store_hilo(ps, hi_slice, lo_slice, hi_engine=None):
                eng = hi_engine or nc.scalar
                if eng is nc.scalar:
                    nc.scalar.copy(hi_slice, ps)
                else:
                    eng.tensor_copy(hi_slice, ps)
                nc.vector.tensor_tensor(
                    out=lo_slice, in0=ps, in1=hi_slice,
                    op=mybir.AluOpType.subtract,
                )

            # ================= K / V / Q projection =================
            with (
                tc.tile_pool(name="pp", bufs=5, space="PSUM") as pp,
                tc.tile_pool(name="pout", bufs=2, space="PSUM") as pout,
                tc.tile_pool(name="pden", bufs=1, space="PSUM") as pden,
            ):
                wk = {c: wp.tile([P, DCP, 2, D], FP8, tag=f"w{c}", name=f"wk{c}") for c in "hl"}

                # ---- K: kt[e, keys] resident ----
                for kb in range(4):
                    xk = {c: xs.tile([P, DCP, 2, 512], FP8, tag="xs", bufs=4, name=f"xk{c}") for c in "hl"}
                    if kb == 0:
                        # critical path: one FIFO queue, consumption order
                        nc.sync.dma_start(wk["h"], w_in["k", "h"][:, :, :, :])
                        nc.sync.dma_start(xk["h"], xk_h[:, :, :, 0:512])
                        nc.sync.dma_start(wk["l"], w_in["k", "l"][:, :, :, :])
                        nc.sync.dma_start(xk["l"], xk_l[:, :, :, 0:512])
                    else:
                        for c in "hl":
                            src = xk_h if c == "h" else xk_l
                            nc.sync.dma_start(
                                xk[c], src[:, :, :, kb * 512 : (kb + 1) * 512]
                            )
                    for ec in range(8):
                        ps = pp.tile([P, 512], F32, tag="ps")
                        proj_psum(
                            ps, wk, xk,
                            lambda w, dcp, ec=ec: w[:, dcp, :, ec * P : (ec + 1) * P],
                            lambda x, dcp: x[:, dcp, :, :],
                        )
                        nc.scalar.copy(
                            kt_h[:, ec // 2, ec % 2, kb * 512 : (kb + 1) * 512], ps
                        )

                # ---- V: v[keys, e] resident ----
                wv = {c: wp.tile([P, DCP, 2, D], FP8, tag=f"w{c}", name=f"wv{c}") for c in "hl"}
                for c in "hl":
                    nc.sync.dma_start(wv[c], w_in["v", c][:, :, :, :])
                for kb in range(4):
                    xv = {c: xs.tile([P, DCP, 2, 512], FP8, tag="xs", bufs=4, name=f"xv{c}") for c in "hl"}
                    for c in "hl":
                        src = xv_h if c == "h" else xv_l
                        nc.sync.dma_start(xv[c], src[:, :, :, kb * 512 : (kb + 1) * 512])
                    for kti in range(4):
                        ktg = kb * 4 + kti
                        for eh in range(2):
                            ps = pp.tile([P, 512], F32, tag="ps")
                            proj_psum(
                                ps, xv, wv,
                                lambda x, dcp, kti=kti: x[:, dcp, :, kti * P : (kti + 1) * P],
                                lambda w, dcp, eh=eh: w[:, dcp, :, eh * 512 : (eh + 1) * 512],
                            )
                            store_hilo(
                                ps,
                                vv["h"][:, ktg, eh * 512 : (eh + 1) * 512],
                                vv["l"][:, ktg, eh * 512 : (eh + 1) * 512],
                            )

                # ---- Q: qt[e, q] resident ----
                wq = {c: wp.tile([P, DCP, 2, D], FP8, tag=f"w{c}", name=f"wq{c}") for c in "hl"}
                for c in "hl":
                    nc.sync.dma_start(wq[c], w_in["q", c][:, :, :, :])
                xq = {c: xs.tile([P, DCP, 2, NQ], FP8, tag=f"xq{c}", bufs=1, name=f"xq{c}") for c in "hl"}
                nc.sync.dma_start(xq["h"], xq_h[:, :, :, :])
                nc.sync.dma_start(xq["l"], xq_l[:, :, :, :])
                for qb in range(2):
                    for ec in range(8):
                        ps = pp.tile([P, 512], F32, tag="ps")
                        proj_psum(
                            ps, wq, xq,
                            lambda w, dcp, ec=ec: w[:, dcp, :, ec * P : (ec + 1) * P],
                            lambda x, dcp, qb=qb: x[:, dcp, :, qb * 512 : (qb + 1) * 512],
                        )
                        nc.scalar.copy(
                            qt_h[:, ec // 2, ec % 2, qb * 512 : (qb + 1) * 512], ps
                        )

                # ============ attention (same PSUM pools stay open) ============

                def score_chunk(t):
                    """S^T[k-chunk t, q] for all slots needing it, + exp hi/lo."""
                    q0 = P * (t // 2)
                    pieces = []
                    pq0 = q0
                    while pq0 < NQ:
                        wp_ = min(512, NQ - pq0)
                        pieces.append((pq0, wp_))
                        pq0 += wp_
                    for pi, (pq0, wp_) in enumerate(pieces):
                        ps = pp.tile([P, 512], F32, tag="ps")
                        for ecp in range(ECP):
                            nc.tensor.matmul(
                                ps[:, :wp_],
                                kt_h[:, ecp, :, t * P : (t + 1) * P],
                                qt_h[:, ecp, :, pq0 : pq0 + wp_],
                                start=(ecp == 0),
                                stop=(ecp == ECP - 1),
                                perf_mode=DR,
                            )
                        if pi == 0:
                            nc.vector.tensor_tensor(
                                out=ps[:, 0:P], in0=ps[:, 0:P],
                                in1=(ma_sb if t % 2 == 0 else mb_sb),
                                op=mybir.AluOpType.add,
                            )
                        tmp = tmpp.tile([P, 512], F32, tag="tmp")
                        nc.scalar.activation(
                            out=tmp[:, :wp_], in_=ps[:, :wp_],
                            func=mybir.ActivationFunctionType.Exp,
                            scale=SCALE_EFF,
                        )
                        store_hilo(
                            tmp[:, :wp_],
                            at["h"][:, t, pq0 : pq0 + wp_],
                            at["l"][:, t, pq0 : pq0 + wp_],
                            hi_engine=nc.gpsimd,
                        )

                def attn_v(s):
                    """out[q-slot s, :] = (a @ 32V) / (a @ 32)."""
                    npair = s + 1
                    ps_den = pden.tile([P, 1], F32, tag="den")
                    n = 0
                    for j in range(npair):
                        for c in "hl":
                            nc.tensor.matmul(
                                ps_den,
                                at[c][:, 2 * j : 2 * j + 2, s * P : (s + 1) * P],
                                ones_sb,
                                start=(n == 0),
                                stop=(n == 2 * npair - 1),
                                perf_mode=DR,
                            )
                            n += 1
                    rec = recp.tile([P, 1], F32, tag="rec")
                    nc.vector.reciprocal(rec, ps_den)
                    out_sb = outp.tile([P, D], BF16, tag="outsb")
                    for eh in range(2):
                        ps_o = pout.tile([P, 512], F32, tag="po")
                        n = 0
                        for j in range(npair):
                            for ca, cv in (("h", "h"), ("h", "l"), ("l", "h")):
                                nc.tensor.matmul(
                                    ps_o,
                                    at[ca][:, 2 * j : 2 * j + 2, s * P : (s + 1) * P],
                                    vv[cv][:, 2 * j : 2 * j + 2, eh * 512 : (eh + 1) * 512],
                                    start=(n == 0),
                                    stop=(n == 3 * npair - 1),
                                    perf_mode=DR,
                                )
                                n += 1
                        # eh0 on ACT, eh1 on DVE: the two copies run in parallel
                        if eh == 0:
                            nc.scalar.activation(
                                out=out_sb[:, 0:512],
                                in_=ps_o,
                                func=mybir.ActivationFunctionType.Copy,
                                scale=rec,
                            )
                        else:
                            nc.vector.tensor_scalar_mul(
                                out_sb[:, 512:1024], ps_o, rec
                            )
                        nc.scalar.dma_start(
                            out[s * P : (s + 1) * P, eh * 512 : (eh + 1) * 512],
                            out_sb[:, eh * 512 : (eh + 1) * 512],
                        )

                # pipeline: score chunks run one slot ahead of attn_v
                nc.scalar.dma_start(ma_sb, mask_a[:, :])
                nc.scalar.dma_start(mb_sb, mask_b[:, :])
                nc.scalar.dma_start(ones_sb, ones_in[:, :, :])
                score_chunk(0)
                score_chunk(1)
                for s in range(NSLOT):
                    if s < NSLOT - 1:
                        score_chunk(2 * s + 2)
                        score_chunk(2 * s + 3)
                    attn_v(s)

    nc.compile()
    return nc


_NC_CACHE = None


def _get_nc():
    global _NC_CACHE
    if _NC_CACHE is None:
        _NC_CACHE = build_attention_nc()
    return _NC_CACHE


def _split8(a):
    import ml_dtypes

    f8 = ml_dtypes.float8_e4m3
    h = a.astype(np.float32).astype(f8)
    l = (a.astype(np.float32) - h.astype(np.float32)).astype(f8)
    return h, l


def _to_xt(a):
    """[rows, d] f32 component -> [128, DCP, 2, rows] fp8 (d on partitions)."""
    rows = a.shape[0]
    return np.ascontiguousarray(a.reshape(rows, DCP, 2, P).transpose(3, 1, 2, 0))


def _to_w(a):
    """[d, e] f32 component -> [128, DCP, 2, e] fp8 (d on partitions)."""
    return np.ascontiguousarray(a.reshape(DCP, 2, P, D).transpose(2, 0, 1, 3))


def _make_masks(h):
    i = np.arange(P)[:, None]
    j = np.arange(P)[None, :]
    if h == 0:
        a = np.where(j >= i, 0.0, NEG).astype(np.float32)  # k partition, q free
        b = np.full((P, P), NEG, dtype=np.float32)
    else:
        a = np.zeros((P, P), dtype=np.float32)
        b = np.where(j >= i, 0.0, NEG).astype(np.float32)
    return a, b


def kernel(
    inputs_for_keys,
    inputs_for_values,
    inputs_for_queries,
    weight_K,
    weight_V,
    weight_Q,
    trace=False,
):
    import ml_dtypes

    xk_full = np.asarray(inputs_for_keys, dtype=np.float32)
    xv_full = np.asarray(inputs_for_values, dtype=np.float32)
    xq_full = np.asarray(inputs_for_queries, dtype=np.float32)

    w_split = {}
    for name, w in (("k", weight_K), ("v", weight_V), ("q", weight_Q)):
        wh, wl = _split8(np.asarray(w, dtype=np.float32) * WSCALE)
        w_split[name] = (_to_w(wh), _to_w(wl))

    xk_split = [tuple(_to_xt(c) for c in _split8(xk_full[b])) for b in range(B)]
    xv_split = [tuple(_to_xt(c) for c in _split8(xv_full[b])) for b in range(B)]

    ones32 = np.full((P, 2, 1), WSCALE, dtype=ml_dtypes.float8_e4m3)
    masks = [_make_masks(0), _make_masks(1)]

    in_maps = []
    for c in range(2 * B):
        b, h = c // 2, c % 2
        rows = np.concatenate(
            [
                xq_full[b, 256 * s + P * h : 256 * s + P * h + P, :]
                for s in range(NSLOT)
            ],
            axis=0,
        )
        qh, ql = _split8(rows)
        ma, mb = masks[h]
        in_maps.append(
            {
                "xk_h": xk_split[b][0], "xk_l": xk_split[b][1],
                "xv_h": xv_split[b][0], "xv_l": xv_split[b][1],
                "xq_h": _to_xt(qh), "xq_l": _to_xt(ql),
                "wk_h": w_split["k"][0], "wk_l": w_split["k"][1],
                "wv_h": w_split["v"][0], "wv_l": w_split["v"][1],
                "wq_h": w_split["q"][0], "wq_l": w_split["q"][1],
                "mask_a": ma, "mask_b": mb,
                "ones32": ones32,
            }
        )

    nc = _get_nc()
    res = bass_utils.run_bass_kernel_spmd(
        nc, in_maps, core_ids=list(range(2 * B)), trace=trace
    )

    out = np.empty((B, S, D), dtype=np.float32)
    for c in range(2 * B):
        b, h = c // 2, c % 2
        o = np.asarray(res.results[c]["out"], dtype=np.float32)
        for s in range(NSLOT):
            out[b, 256 * s + P * h : 256 * s + P * h + P, :] = o[
                s * P : (s + 1) * P, :
            ]

    if trace:
        return out, res
    return out


# revision 21
# speedup vs baseline: 2.0370x; 1.1422x over previous
"""Causal single-head attention (B=4, S=2048, D=1024) on 8 TRN2 NeuronCores.

Sharding: core c -> (batch b = c//2, half h = c%2). Every core runs the SAME
program: its 1024 query rows are 8 slots of 128 rows; slot s holds global
rows [256*s + 128*h, 256*s + 128*h + 128) of batch b, whose padded causal
key-length is 256*(s+1).

All matmuls run as fp8(e4m3) DoubleRow (2 contraction tiles per instruction,
0.5 cycles/row) with 3-term hi/lo error compensation:
    x @ w ~= xh@wh + xh@wl + xl@wh        (drop xl@wl, ~0.1% error)
X^T and W (pre-scaled by 32) are split hi/lo on the host and shipped fp8, so
the kernel needs no PE transposes for inputs. Scores are computed transposed
(S^T[k, q] with keys on partitions) so the attention weights can be consumed
directly as DoubleRow stationaries by attn @ V -- no attention transposes
either. exp() output is split hi/lo on device (ACT copy + DVE subtract).
The softmax denominator comes from an extra ones-column DoubleRow matmul and
cancels the 32x V scale exactly. The causal mask is one 128x128 f32 add per
key-chunk (two host mask tiles, selected by parity).
"""

import numpy as np

import concourse.bacc as bacc
import concourse.mybir as mybir
import concourse.tile as tile
from concourse import bass_utils

B, S, D = 4, 2048, 1024
P = 128
DCP = 4              # pairs of 128-deep contraction tiles (d dim)
ECP = 4              # pairs of 128-wide e tiles
NSLOT = 8            # q tiles per core
NQ = NSLOT * P       # 1024 q rows per core
NKC = S // P         # 16 key chunks
WSCALE = 32.0        # host pre-scale on all three weights
SCALE_EFF = 1.0 / (WSCALE * WSCALE * float(np.sqrt(np.float32(S))))
NEG = -1.0e9

F32 = mybir.dt.float32
FP8 = mybir.dt.float8e4
BF16 = mybir.dt.bfloat16
DR = mybir.MatmulPerfMode.DoubleRow


def build_attention_nc():
    nc = bacc.Bacc("TRN2", target_bir_lowering=False)

    xk_h = nc.dram_tensor("xk_h", [P, DCP, 2, S], FP8, kind="ExternalInput")
    xk_l = nc.dram_tensor("xk_l", [P, DCP, 2, S], FP8, kind="ExternalInput")
    xv_h = nc.dram_tensor("xv_h", [P, DCP, 2, S], FP8, kind="ExternalInput")
    xv_l = nc.dram_tensor("xv_l", [P, DCP, 2, S], FP8, kind="ExternalInput")
    xq_h = nc.dram_tensor("xq_h", [P, DCP, 2, NQ], FP8, kind="ExternalInput")
    xq_l = nc.dram_tensor("xq_l", [P, DCP, 2, NQ], FP8, kind="ExternalInput")
    w_in = {}
    for t in ("k", "v", "q"):
        comps = ("h", "l") if t == "v" else ("h",)
        for c in comps:
            w_in[t, c] = nc.dram_tensor(
                f"w{t}_{c}", [P, DCP, 2, D], FP8, kind="ExternalInput"
            )
    mask_a = nc.dram_tensor("mask_a", [P, P], F32, kind="ExternalInput")
    mask_b = nc.dram_tensor("mask_b", [P, P], F32, kind="ExternalInput")
    ones_in = nc.dram_tensor("ones32", [P, 2, 1], FP8, kind="ExternalInput")
    out = nc.dram_tensor("out", [NQ, D], BF16, kind="ExternalOutput")

    with tile.TileContext(nc) as tc:
        with (
            tc.tile_pool(name="res", bufs=1) as res,
            tc.tile_pool(name="wp", bufs=2) as wp,
            tc.tile_pool(name="xs", bufs=2) as xs,
            tc.tile_pool(name="tmpp", bufs=3) as tmpp,
            tc.tile_pool(name="outp", bufs=2) as outp,
            tc.tile_pool(name="recp", bufs=2) as recp,
        ):
            # scores run 1-term (hi only): kt/qt keep no lo component
            kt_h = res.tile([P, ECP, 2, S], FP8, tag="kth", name="kt_h")
            vv = {c: res.tile([P, NKC, D], FP8, tag=f"v{c}", name=f"v{c}") for c in "hl"}
            qt_h = res.tile([P, ECP, 2, NQ], FP8, tag="qth", name="qt_h")
            at = {c: res.tile([P, NKC, NQ], FP8, tag=f"at{c}", name=f"at{c}") for c in "hl"}
            ma_sb = res.tile([P, P], F32, tag="maska")
            mb_sb = res.tile([P, P], F32, tag="maskb")
            ones_sb = res.tile([P, 2, 1], FP8, tag="ones")

            def proj_psum(ps, stats, movs, stat_slice, mov_slice, terms):
                """len(terms) x 4 DR matmuls accumulating one psum group."""
                total = len(terms) * DCP
                n = 0
                for cs, cm in terms:
                    for dcp in range(DCP):
                        nc.tensor.matmul(
                            ps,
                            stat_slice(stats[cs], dcp),
                            mov_slice(movs[cm], dcp),
                            start=(n == 0),
                            stop=(n == total - 1),
                            perf_mode=DR,
                        )
                        n += 1

            def store_hilo(ps, hi_slice, lo_slice, hi_engine=None):
                eng = hi_engine or nc.scalar
                if eng is nc.scalar:
                    nc.scalar.copy(hi_slice, ps)
                else:
                    eng.tensor_copy(hi_slice, ps)
                nc.vector.tensor_tensor(
                    out=lo_slice, in0=ps, in1=hi_slice,
                    op=mybir.AluOpType.subtract,
                )

            # ================= K / V / Q projection =================
            with (
                tc.tile_pool(name="pp", bufs=5, space="PSUM") as pp,
                tc.tile_pool(name="pout", bufs=2, space="PSUM") as pout,
                tc.tile_pool(name="pden", bufs=1, space="PSUM") as pden,
            ):
                wk = {"h": wp.tile([P, DCP, 2, D], FP8, tag="wh", name="wkh")}

                # ---- K: kt[e, keys] resident (2-term: x exact, w hi) ----
                for kb in range(4):
                    xk = {c: xs.tile([P, DCP, 2, 512], FP8, tag="xs", bufs=4, name=f"xk{c}") for c in "hl"}
                    if kb == 0:
                        # critical path: one FIFO queue, consumption order
                        nc.sync.dma_start(wk["h"], w_in["k", "h"][:, :, :, :])
                        nc.sync.dma_start(xk["h"], xk_h[:, :, :, 0:512])
                        nc.sync.dma_start(xk["l"], xk_l[:, :, :, 0:512])
                    else:
                        for c in "hl":
                            src = xk_h if c == "h" else xk_l
                            nc.sync.dma_start(
                                xk[c], src[:, :, :, kb * 512 : (kb + 1) * 512]
                            )
                    for ec in range(8):
                        ps = pp.tile([P, 512], F32, tag="ps")
                        proj_psum(
                            ps, wk, xk,
                            lambda w, dcp, ec=ec: w[:, dcp, :, ec * P : (ec + 1) * P],
                            lambda x, dcp: x[:, dcp, :, :],
                            terms=(("h", "h"), ("h", "l")),
                        )
                        nc.scalar.copy(
                            kt_h[:, ec // 2, ec % 2, kb * 512 : (kb + 1) * 512], ps
                        )

                # ---- V: v[keys, e] resident ----
                wv = {c: wp.tile([P, DCP, 2, D], FP8, tag=f"w{c}", name=f"wv{c}") for c in "hl"}
                # (V keeps full 3-term compensation)
                for c in "hl":
                    nc.sync.dma_start(wv[c], w_in["v", c][:, :, :, :])
                for kb in range(4):
                    xv = {c: xs.tile([P, DCP, 2, 512], FP8, tag="xs", bufs=4, name=f"xv{c}") for c in "hl"}
                    for c in "hl":
                        src = xv_h if c == "h" else xv_l
                        nc.sync.dma_start(xv[c], src[:, :, :, kb * 512 : (kb + 1) * 512])
                    for kti in range(4):
                        ktg = kb * 4 + kti
                        for eh in range(2):
                            ps = pp.tile([P, 512], F32, tag="ps")
                            proj_psum(
                                ps, xv, wv,
                                lambda x, dcp, kti=kti: x[:, dcp, :, kti * P : (kti + 1) * P],
                                lambda w, dcp, eh=eh: w[:, dcp, :, eh * 512 : (eh + 1) * 512],
                                terms=(("h", "h"), ("h", "l"), ("l", "h")),
                            )
                            store_hilo(
                                ps,
                                vv["h"][:, ktg, eh * 512 : (eh + 1) * 512],
                                vv["l"][:, ktg, eh * 512 : (eh + 1) * 512],
                            )

                # ---- Q: qt[e, q] resident ----
                wq = {"h": wp.tile([P, DCP, 2, D], FP8, tag="wh", name="wqh")}
                nc.sync.dma_start(wq["h"], w_in["q", "h"][:, :, :, :])
                xq = {c: xs.tile([P, DCP, 2, NQ], FP8, tag=f"xq{c}", bufs=1, name=f"xq{c}") for c in "hl"}
                nc.sync.dma_start(xq["h"], xq_h[:, :, :, :])
                nc.sync.dma_start(xq["l"], xq_l[:, :, :, :])
                for qb in range(2):
                    for ec in range(8):
                        ps = pp.tile([P, 512], F32, tag="ps")
                        proj_psum(
                            ps, wq, xq,
                            lambda w, dcp, ec=ec: w[:, dcp, :, ec * P : (ec + 1) * P],
                            lambda x, dcp, qb=qb: x[:, dcp, :, qb * 512 : (qb + 1) * 512],
                            terms=(("h", "h"), ("h", "l")),
                        )
                        nc.scalar.copy(
                            qt_h[:, ec // 2, ec % 2, qb * 512 : (qb + 1) * 512], ps
                        )

                # ============ attention (same PSUM pools stay open) ============

                def score_chunk(t):
                    """S^T[k-chunk t, q] for all slots needing it, + exp hi/lo."""
                    q0 = P * (t // 2)
                    pieces = [(q0, P)]
                    pq0 = q0 + P
                    while pq0 < NQ:
                        wp_ = min(512, NQ - pq0)
                        pieces.append((pq0, wp_))
                        pq0 += wp_
                    for pi, (pq0, wp_) in enumerate(pieces):
                        ps = pp.tile([P, 512], F32, tag="ps")
                        for ecp in range(ECP):
                            nc.tensor.matmul(
                                ps[:, :wp_],
                                kt_h[:, ecp, :, t * P : (t + 1) * P],
                                qt_h[:, ecp, :, pq0 : pq0 + wp_],
                                start=(ecp == 0),
                                stop=(ecp == ECP - 1),
                                perf_mode=DR,
                            )
                        if pi == 0:
                            nc.vector.tensor_tensor(
                                out=ps[:, 0:P], in0=ps[:, 0:P],
                                in1=(ma_sb if t % 2 == 0 else mb_sb),
                                op=mybir.AluOpType.add,
                            )
                        tmp = tmpp.tile([P, 512], F32, tag="tmp")
                        nc.scalar.activation(
                            out=tmp[:, :wp_], in_=ps[:, :wp_],
                            func=mybir.ActivationFunctionType.Exp,
                            scale=SCALE_EFF,
                        )
                        store_hilo(
                            tmp[:, :wp_],
                            at["h"][:, t, pq0 : pq0 + wp_],
                            at["l"][:, t, pq0 : pq0 + wp_],
                            hi_engine=nc.gpsimd,
                        )

                def attn_v(s):
                    """out[q-slot s, :] = (a @ 32V) / (a @ 32)."""
                    npair = s + 1
                    ps_den = pden.tile([P, 1], F32, tag="den")
                    n = 0
                    for j in range(npair):
                        for c in "hl":
                            nc.tensor.matmul(
                                ps_den,
                                at[c][:, 2 * j : 2 * j + 2, s * P : (s + 1) * P],
                                ones_sb,
                                start=(n == 0),
                                stop=(n == 2 * npair - 1),
                                perf_mode=DR,
                            )
                            n += 1
                    rec = recp.tile([P, 1], F32, tag="rec")
                    nc.vector.reciprocal(rec, ps_den)
                    out_sb = outp.tile([P, D], BF16, tag="outsb")
                    for eh in range(2):
                        ps_o = pout.tile([P, 512], F32, tag="po")
                        n = 0
                        for j in range(npair):
                            for ca, cv in (("h", "h"), ("h", "l"), ("l", "h")):
                                nc.tensor.matmul(
                                    ps_o,
                                    at[ca][:, 2 * j : 2 * j + 2, s * P : (s + 1) * P],
                                    vv[cv][:, 2 * j : 2 * j + 2, eh * 512 : (eh + 1) * 512],
                                    start=(n == 0),
                                    stop=(n == 3 * npair - 1),
                                    perf_mode=DR,
                                )
                                n += 1
                        # eh0 on ACT, eh1 on DVE: the two copies run in parallel
                        if eh == 0:
                            nc.scalar.activation(
                                out=out_sb[:, 0:512],
                                in_=ps_o,
                                func=mybir.ActivationFunctionType.Copy,
                                scale=rec,
                            )
                        else:
                            nc.vector.tensor_scalar_mul(
                                out_sb[:, 512:1024], ps_o, rec
                            )
                        oq = nc.sync if eh == 0 else nc.scalar
                        oq.dma_start(
                            out[s * P : (s + 1) * P, eh * 512 : (eh + 1) * 512],
                            out_sb[:, eh * 512 : (eh + 1) * 512],
                        )

                # pipeline: score chunks run one slot ahead of attn_v
                nc.scalar.dma_start(ma_sb, mask_a[:, :])
                nc.scalar.dma_start(mb_sb, mask_b[:, :])
                nc.scalar.dma_start(ones_sb, ones_in[:, :, :])
                score_chunk(0)
                score_chunk(1)
                for s in range(NSLOT):
                    if s < NSLOT - 1:
                        score_chunk(2 * s + 2)
                        score_chunk(2 * s + 3)
                    attn_v(s)

    nc.compile()
    return nc


_NC_CACHE = None


def _get_nc():
    global _NC_CACHE
    if _NC_CACHE is None:
        _NC_CACHE = build_attention_nc()
    return _NC_CACHE


def _split8(a):
    import ml_dtypes

    f8 = ml_dtypes.float8_e4m3
    h = a.astype(np.float32).astype(f8)
    l = (a.astype(np.float32) - h.astype(np.float32)).astype(f8)
    return h, l


def _to_xt(a):
    """[rows, d] f32 component -> [128, DCP, 2, rows] fp8 (d on partitions)."""
    rows = a.shape[0]
    return np.ascontiguousarray(a.reshape(rows, DCP, 2, P).transpose(3, 1, 2, 0))


def _to_w(a):
    """[d, e] f32 component -> [128, DCP, 2, e] fp8 (d on partitions)."""
    return np.ascontiguousarray(a.reshape(DCP, 2, P, D).transpose(2, 0, 1, 3))


def _make_masks(h):
    i = np.arange(P)[:, None]
    j = np.arange(P)[None, :]
    if h == 0:
        a = np.where(j >= i, 0.0, NEG).astype(np.float32)  # k partition, q free
        b = np.full((P, P), NEG, dtype=np.float32)
    else:
        a = np.zeros((P, P), dtype=np.float32)
        b = np.where(j >= i, 0.0, NEG).astype(np.float32)
    return a, b


def kernel(
    inputs_for_keys,
    inputs_for_values,
    inputs_for_queries,
    weight_K,
    weight_V,
    weight_Q,
    trace=False,
):
    import ml_dtypes

    xk_full = np.asarray(inputs_for_keys, dtype=np.float32)
    xv_full = np.asarray(inputs_for_values, dtype=np.float32)
    xq_full = np.asarray(inputs_for_queries, dtype=np.float32)

    w_split = {}
    for name, w in (("k", weight_K), ("v", weight_V), ("q", weight_Q)):
        wh, wl = _split8(np.asarray(w, dtype=np.float32) * WSCALE)
        w_split[name] = (_to_w(wh), _to_w(wl))
    # K/Q projections run 2-term (w-hi only); only V ships its lo component

    xk_split = [tuple(_to_xt(c) for c in _split8(xk_full[b])) for b in range(B)]
    xv_split = [tuple(_to_xt(c) for c in _split8(xv_full[b])) for b in range(B)]

    ones32 = np.full((P, 2, 1), WSCALE, dtype=ml_dtypes.float8_e4m3)
    masks = [_make_masks(0), _make_masks(1)]

    in_maps = []
    for c in range(2 * B):
        b, h = c // 2, c % 2
        rows = np.concatenate(
            [
                xq_full[b, 256 * s + P * h : 256 * s + P * h + P, :]
                for s in range(NSLOT)
            ],
            axis=0,
        )
        qh, ql = _split8(rows)
        ma, mb = masks[h]
        in_maps.append(
            {
                "xk_h": xk_split[b][0], "xk_l": xk_split[b][1],
                "xv_h": xv_split[b][0], "xv_l": xv_split[b][1],
                "xq_h": _to_xt(qh), "xq_l": _to_xt(ql),
                "wk_h": w_split["k"][0],
                "wv_h": w_split["v"][0], "wv_l": w_split["v"][1],
                "wq_h": w_split["q"][0],
                "mask_a": ma, "mask_b": mb,
                "ones32": ones32,
            }
        )

    nc = _get_nc()
    res = bass_utils.run_bass_kernel_spmd(
        nc, in_maps, core_ids=list(range(2 * B)), trace=trace
    )

    out = np.empty((B, S, D), dtype=np.float32)
    for c in range(2 * B):
        b, h = c // 2, c % 2
        o = np.asarray(res.results[c]["out"], dtype=np.float32)
        for s in range(NSLOT):
            out[b, 256 * s + P * h : 256 * s + P * h + P, :] = o[
                s * P : (s + 1) * P, :
            ]

    if trace:
        return out, res
    return out


# revision 25
# speedup vs baseline: 2.0860x; 1.0241x over previous
"""Causal single-head attention (B=4, S=2048, D=1024) on 8 TRN2 NeuronCores.

Sharding: core c -> (batch b = c//2, half h = c%2). Every core runs the SAME
program: its 1024 query rows are 8 slots of 128 rows; slot s holds global
rows [256*s + 128*h, 256*s + 128*h + 128) of batch b, whose padded causal
key-length is 256*(s+1).

All matmuls run as fp8(e4m3) DoubleRow (2 contraction tiles per instruction,
0.5 cycles/row) with 3-term hi/lo error compensation:
    x @ w ~= xh@wh + xh@wl + xl@wh        (drop xl@wl, ~0.1% error)
X^T and W (pre-scaled by 32) are split hi/lo on the host and shipped fp8, so
the kernel needs no PE transposes for inputs. Scores are computed transposed
(S^T[k, q] with keys on partitions) so the attention weights can be consumed
directly as DoubleRow stationaries by attn @ V -- no attention transposes
either. exp() output is split hi/lo on device (ACT copy + DVE subtract).
The softmax denominator comes from an extra ones-column DoubleRow matmul and
cancels the 32x V scale exactly. The causal mask is one 128x128 f32 add per
key-chunk (two host mask tiles, selected by parity).
"""

import numpy as np

import concourse.bacc as bacc
import concourse.mybir as mybir
import concourse.tile as tile
from concourse import bass_utils

B, S, D = 4, 2048, 1024
P = 128
DCP = 4              # pairs of 128-deep contraction tiles (d dim)
ECP = 4              # pairs of 128-wide e tiles
NSLOT = 8            # q tiles per core
NQ = NSLOT * P       # 1024 q rows per core
NKC = S // P         # 16 key chunks
WSCALE = 32.0        # host pre-scale on all three weights
SCALE_EFF = 1.0 / (WSCALE * WSCALE * float(np.sqrt(np.float32(S))))
NEG = -1.0e9

F32 = mybir.dt.float32
FP8 = mybir.dt.float8e4
BF16 = mybir.dt.bfloat16
DR = mybir.MatmulPerfMode.DoubleRow


def build_attention_nc():
    nc = bacc.Bacc("TRN2", target_bir_lowering=False)

    xk_h = nc.dram_tensor("xk_h", [P, DCP, 2, S], FP8, kind="ExternalInput")
    xk_l = nc.dram_tensor("xk_l", [P, DCP, 2, S], FP8, kind="ExternalInput")
    xv_h = nc.dram_tensor("xv_h", [P, DCP, 2, S], FP8, kind="ExternalInput")
    xv_l = nc.dram_tensor("xv_l", [P, DCP, 2, S], FP8, kind="ExternalInput")
    xq_h = nc.dram_tensor("xq_h", [P, DCP, 2, NQ], FP8, kind="ExternalInput")
    xq_l = nc.dram_tensor("xq_l", [P, DCP, 2, NQ], FP8, kind="ExternalInput")
    w_in = {}
    for t in ("k", "v", "q"):
        comps = ("h", "l") if t == "v" else ("h",)
        for c in comps:
            w_in[t, c] = nc.dram_tensor(
                f"w{t}_{c}", [P, DCP, 2, D], FP8, kind="ExternalInput"
            )
    mask_a = nc.dram_tensor("mask_a", [P, P], F32, kind="ExternalInput")
    mask_b = nc.dram_tensor("mask_b", [P, P], F32, kind="ExternalInput")
    ones_in = nc.dram_tensor("ones32", [P, 2, 1], FP8, kind="ExternalInput")
    out = nc.dram_tensor("out", [NQ, D], BF16, kind="ExternalOutput")

    with tile.TileContext(nc) as tc:
        with (
            tc.tile_pool(name="res", bufs=1) as res,
            tc.tile_pool(name="wp", bufs=2) as wp,
            tc.tile_pool(name="xs", bufs=2) as xs,
            tc.tile_pool(name="tmpp", bufs=3) as tmpp,
            tc.tile_pool(name="outp", bufs=2) as outp,
            tc.tile_pool(name="recp", bufs=2) as recp,
        ):
            # scores run 1-term (hi only): kt/qt keep no lo component
            kt_h = res.tile([P, ECP, 2, S], FP8, tag="kth", name="kt_h")
            vv = {c: res.tile([P, NKC, D], FP8, tag=f"v{c}", name=f"v{c}") for c in "hl"}
            qt_h = res.tile([P, ECP, 2, NQ], FP8, tag="qth", name="qt_h")
            at = {c: res.tile([P, NKC, NQ], FP8, tag=f"at{c}", name=f"at{c}") for c in "hl"}
            ma_sb = res.tile([P, P], F32, tag="maska")
            mb_sb = res.tile([P, P], F32, tag="maskb")
            ones_sb = res.tile([P, 2, 1], FP8, tag="ones")

            def proj_psum(ps, stats, movs, stat_slice, mov_slice, terms):
                """len(terms) x 4 DR matmuls accumulating one psum group."""
                total = len(terms) * DCP
                n = 0
                for cs, cm in terms:
                    for dcp in range(DCP):
                        nc.tensor.matmul(
                            ps,
                            stat_slice(stats[cs], dcp),
                            mov_slice(movs[cm], dcp),
                            start=(n == 0),
                            stop=(n == total - 1),
                            perf_mode=DR,
                        )
                        n += 1

            def store_hilo(ps, hi_slice, lo_slice, hi_engine=None):
                eng = hi_engine or nc.scalar
                if eng is nc.scalar:
                    nc.scalar.copy(hi_slice, ps)
                else:
                    eng.tensor_copy(hi_slice, ps)
                nc.vector.tensor_tensor(
                    out=lo_slice, in0=ps, in1=hi_slice,
                    op=mybir.AluOpType.subtract,
                )

            # ================= K / V / Q projection =================
            with (
                tc.tile_pool(name="pp", bufs=6, space="PSUM") as pp,
                tc.tile_pool(name="pout", bufs=2, space="PSUM") as pout,
            ):
                wk = {"h": wp.tile([P, DCP, 2, D], FP8, tag="wh", name="wkh")}

                # ---- K: kt[e, keys] resident (2-term: x exact, w hi) ----
                for kb in range(4):
                    xk = {c: xs.tile([P, DCP, 2, 512], FP8, tag="xs", bufs=4, name=f"xk{c}") for c in "hl"}
                    if kb == 0:
                        # critical path: one FIFO queue, consumption order
                        nc.sync.dma_start(
                            wk["h"][:, 0:2, :, :], w_in["k", "h"][:, 0:2, :, :]
                        )
                        nc.sync.dma_start(
                            xk["h"][:, 0:2, :, :], xk_h[:, 0:2, :, 0:512]
                        )
                        nc.sync.dma_start(
                            wk["h"][:, 2:4, :, :], w_in["k", "h"][:, 2:4, :, :]
                        )
                        nc.sync.dma_start(
                            xk["h"][:, 2:4, :, :], xk_h[:, 2:4, :, 0:512]
                        )
                        nc.sync.dma_start(xk["l"], xk_l[:, :, :, 0:512])
                    else:
                        for c in "hl":
                            src = xk_h if c == "h" else xk_l
                            nc.sync.dma_start(
                                xk[c], src[:, :, :, kb * 512 : (kb + 1) * 512]
                            )
                    for ec in range(8):
                        ps = pp.tile([P, 512], F32, tag="ps")
                        proj_psum(
                            ps, wk, xk,
                            lambda w, dcp, ec=ec: w[:, dcp, :, ec * P : (ec + 1) * P],
                            lambda x, dcp: x[:, dcp, :, :],
                            terms=(("h", "h"), ("h", "l")),
                        )
                        nc.scalar.copy(
                            kt_h[:, ec // 2, ec % 2, kb * 512 : (kb + 1) * 512], ps
                        )

                # ---- V: v[keys, e] resident ----
                wv = {c: wp.tile([P, DCP, 2, D], FP8, tag=f"w{c}", name=f"wv{c}") for c in "hl"}
                # (V keeps full 3-term compensation)
                for c in "hl":
                    nc.sync.dma_start(wv[c], w_in["v", c][:, :, :, :])
                for kb in range(4):
                    xv = {c: xs.tile([P, DCP, 2, 512], FP8, tag="xs", bufs=4, name=f"xv{c}") for c in "hl"}
                    for c in "hl":
                        src = xv_h if c == "h" else xv_l
                        nc.sync.dma_start(xv[c], src[:, :, :, kb * 512 : (kb + 1) * 512])
                    for kti in range(4):
                        ktg = kb * 4 + kti
                        for eh in range(2):
                            ps = pp.tile([P, 512], F32, tag="ps")
                            proj_psum(
                                ps, xv, wv,
                                lambda x, dcp, kti=kti: x[:, dcp, :, kti * P : (kti + 1) * P],
                                lambda w, dcp, eh=eh: w[:, dcp, :, eh * 512 : (eh + 1) * 512],
                                terms=(("h", "h"), ("h", "l"), ("l", "h")),
                            )
                            store_hilo(
                                ps,
                                vv["h"][:, ktg, eh * 512 : (eh + 1) * 512],
                                vv["l"][:, ktg, eh * 512 : (eh + 1) * 512],
                            )

                # ---- Q: qt[e, q] resident ----
                wq = {"h": wp.tile([P, DCP, 2, D], FP8, tag="wh", name="wqh")}
                nc.sync.dma_start(wq["h"], w_in["q", "h"][:, :, :, :])
                xq = {c: xs.tile([P, DCP, 2, NQ], FP8, tag=f"xq{c}", bufs=1, name=f"xq{c}") for c in "hl"}
                nc.sync.dma_start(xq["h"], xq_h[:, :, :, :])
                nc.sync.dma_start(xq["l"], xq_l[:, :, :, :])
                for qb in range(2):
                    for ec in range(8):
                        ps = pp.tile([P, 512], F32, tag="ps")
                        proj_psum(
                            ps, wq, xq,
                            lambda w, dcp, ec=ec: w[:, dcp, :, ec * P : (ec + 1) * P],
                            lambda x, dcp, qb=qb: x[:, dcp, :, qb * 512 : (qb + 1) * 512],
                            terms=(("h", "h"), ("h", "l")),
                        )
                        nc.scalar.copy(
                            qt_h[:, ec // 2, ec % 2, qb * 512 : (qb + 1) * 512], ps
                        )

                # ============ attention (same PSUM pools stay open) ============

                def score_chunk(t):
                    """S^T[k-chunk t, q] for all slots needing it, + exp hi/lo."""
                    q0 = P * (t // 2)
                    pieces = [(q0, P)]
                    pq0 = q0 + P
                    while pq0 < NQ:
                        wp_ = min(512, NQ - pq0)
                        pieces.append((pq0, wp_))
                        pq0 += wp_
                    for pi, (pq0, wp_) in enumerate(pieces):
                        ps = pp.tile([P, 512], F32, tag="ps")
                        for ecp in range(ECP):
                            nc.tensor.matmul(
                                ps[:, :wp_],
                                kt_h[:, ecp, :, t * P : (t + 1) * P],
                                qt_h[:, ecp, :, pq0 : pq0 + wp_],
                                start=(ecp == 0),
                                stop=(ecp == ECP - 1),
                                perf_mode=DR,
                            )
                        if pi == 0:
                            nc.vector.tensor_tensor(
                                out=ps[:, 0:P], in0=ps[:, 0:P],
                                in1=(ma_sb if t % 2 == 0 else mb_sb),
                                op=mybir.AluOpType.add,
                            )
                        tmp = tmpp.tile([P, 512], F32, tag="tmp")
                        nc.scalar.activation(
                            out=tmp[:, :wp_], in_=ps[:, :wp_],
                            func=mybir.ActivationFunctionType.Exp,
                            scale=SCALE_EFF,
                        )
                        store_hilo(
                            tmp[:, :wp_],
                            at["h"][:, t, pq0 : pq0 + wp_],
                            at["l"][:, t, pq0 : pq0 + wp_],
                            hi_engine=nc.gpsimd,
                        )

                def attn_v_tail_group(s, ps_o, rec, out_sb, lo, hi, npair):
                    n = 0
                    w = hi - lo
                    for j in range(npair):
                        for ca, cv in (("h", "h"), ("h", "l"), ("l", "h")):
                            nc.tensor.matmul(
                                ps_o[:, 0 : w],
                                at[ca][:, 2 * j : 2 * j + 2, s * P : (s + 1) * P],
                                vv[cv][:, 2 * j : 2 * j + 2, lo:hi],
                                start=(n == 0),
                                stop=(n == 3 * npair - 1),
                                perf_mode=DR,
                            )
                            n += 1

                def attn_v(s):
                    """out[q-slot s, :] = (a @ 32V) / (a @ 32)."""
                    npair = s + 1
                    ps_den_t = pp.tile([P, 512], F32, tag="ps", name="ps_den")
                    ps_den = ps_den_t[:, 0:1]
                    n = 0
                    for j in range(npair):
                        for c in "hl":
                            nc.tensor.matmul(
                                ps_den,
                                at[c][:, 2 * j : 2 * j + 2, s * P : (s + 1) * P],
                                ones_sb,
                                start=(n == 0),
                                stop=(n == 2 * npair - 1),
                                perf_mode=DR,
                            )
                            n += 1
                    rec = recp.tile([P, 1], F32, tag="rec")
                    nc.vector.reciprocal(rec, ps_den)
                    out_sb = outp.tile([P, D], BF16, tag="outsb")
                    # the last slot splits eh1 into two 256-wide psum groups so
                    # its copy/DMA pipeline hides under real matmuls
                    groups = [(0, 512), (512, 1024)]
                    if s == NSLOT - 1:
                        groups = [(0, 512), (512, 768), (768, 1024)]
                    for gi, (lo, hi) in enumerate(groups):
                        ps_o = pout.tile([P, 512], F32, tag="po")
                        attn_v_tail_group(s, ps_o, rec, out_sb, lo, hi, npair)
                        eng = nc.scalar if gi % 2 == 0 else nc.vector
                        if eng is nc.scalar:
                            nc.scalar.activation(
                                out=out_sb[:, lo:hi],
                                in_=ps_o[:, 0 : hi - lo],
                                func=mybir.ActivationFunctionType.Copy,
                                scale=rec,
                            )
                        else:
                            nc.vector.tensor_scalar_mul(
                                out_sb[:, lo:hi], ps_o[:, 0 : hi - lo], rec
                            )
                        oq = nc.sync if gi % 2 == 0 else nc.scalar
                        oq.dma_start(
                            out[s * P : (s + 1) * P, lo:hi], out_sb[:, lo:hi]
                        )

                # pipeline: score chunks run one slot ahead of attn_v
                nc.scalar.dma_start(ma_sb, mask_a[:, :])
                nc.scalar.dma_start(mb_sb, mask_b[:, :])
                nc.scalar.dma_start(ones_sb, ones_in[:, :, :])
                score_chunk(0)
                score_chunk(1)
                for s in range(NSLOT):
                    if s < NSLOT - 1:
                        score_chunk(2 * s + 2)
                        score_chunk(2 * s + 3)
                    attn_v(s)

    nc.compile()
    return nc


_NC_CACHE = None


def _get_nc():
    global _NC_CACHE
    if _NC_CACHE is None:
        _NC_CACHE = build_attention_nc()
    return _NC_CACHE


def _split8(a):
    import ml_dtypes

    f8 = ml_dtypes.float8_e4m3
    h = a.astype(np.float32).astype(f8)
    l = (a.astype(np.float32) - h.astype(np.float32)).astype(f8)
    return h, l


def _to_xt(a):
    """[rows, d] f32 component -> [128, DCP, 2, rows] fp8 (d on partitions)."""
    rows = a.shape[0]
    return np.ascontiguousarray(a.reshape(rows, DCP, 2, P).transpose(3, 1, 2, 0))


def _to_w(a):
    """[d, e] f32 component -> [128, DCP, 2, e] fp8 (d on partitions)."""
    return np.ascontiguousarray(a.reshape(DCP, 2, P, D).transpose(2, 0, 1, 3))


def _make_masks(h):
    i = np.arange(P)[:, None]
    j = np.arange(P)[None, :]
    if h == 0:
        a = np.where(j >= i, 0.0, NEG).astype(np.float32)  # k partition, q free
        b = np.full((P, P), NEG, dtype=np.float32)
    else:
        a = np.zeros((P, P), dtype=np.float32)
        b = np.where(j >= i, 0.0, NEG).astype(np.float32)
    return a, b


def kernel(
    inputs_for_keys,
    inputs_for_values,
    inputs_for_queries,
    weight_K,
    weight_V,
    weight_Q,
    trace=False,
):
    import ml_dtypes

    xk_full = np.asarray(inputs_for_keys, dtype=np.float32)
    xv_full = np.asarray(inputs_for_values, dtype=np.float32)
    xq_full = np.asarray(inputs_for_queries, dtype=np.float32)

    w_split = {}
    for name, w in (("k", weight_K), ("v", weight_V), ("q", weight_Q)):
        wh, wl = _split8(np.asarray(w, dtype=np.float32) * WSCALE)
        w_split[name] = (_to_w(wh), _to_w(wl))
    # K/Q projections run 2-term (w-hi only); only V ships its lo component

    xk_split = [tuple(_to_xt(c) for c in _split8(xk_full[b])) for b in range(B)]
    xv_split = [tuple(_to_xt(c) for c in _split8(xv_full[b])) for b in range(B)]

    ones32 = np.full((P, 2, 1), WSCALE, dtype=ml_dtypes.float8_e4m3)
    masks = [_make_masks(0), _make_masks(1)]

    in_maps = []
    for c in range(2 * B):
        b, h = c // 2, c % 2
        rows = np.concatenate(
            [
                xq_full[b, 256 * s + P * h : 256 * s + P * h + P, :]
                for s in range(NSLOT)
            ],
            axis=0,
        )
        qh, ql = _split8(rows)
        ma, mb = masks[h]
        in_maps.append(
            {
                "xk_h": xk_split[b][0], "xk_l": xk_split[b][1],
                "xv_h": xv_split[b][0], "xv_l": xv_split[b][1],
                "xq_h": _to_xt(qh), "xq_l": _to_xt(ql),
                "wk_h": w_split["k"][0],
                "wv_h": w_split["v"][0], "wv_l": w_split["v"][1],
                "wq_h": w_split["q"][0],
                "mask_a": ma, "mask_b": mb,
                "ones32": ones32,
            }
        )

    nc = _get_nc()
    res = bass_utils.run_bass_kernel_spmd(
        nc, in_maps, core_ids=list(range(2 * B)), trace=trace
    )

    out = np.empty((B, S, D), dtype=np.float32)
    for c in range(2 * B):
        b, h = c // 2, c % 2
        o = np.asarray(res.results[c]["out"], dtype=np.float32)
        for s in range(NSLOT):
            out[b, 256 * s + P * h : 256 * s + P * h + P, :] = o[
                s * P : (s + 1) * P, :
            ]

    if trace:
        return out, res
    return out


# revision 28
# speedup vs baseline: 2.1140x; 1.0134x over previous
"""Causal single-head attention (B=4, S=2048, D=1024) on 8 TRN2 NeuronCores.

Sharding: core c -> (batch b = c//2, half h = c%2). Every core runs the SAME
program: its 1024 query rows are 8 slots of 128 rows; slot s holds global
rows [256*s + 128*h, 256*s + 128*h + 128) of batch b, whose padded causal
key-length is 256*(s+1).

All matmuls run as fp8(e4m3) DoubleRow (2 contraction tiles per instruction,
0.5 cycles/row) with 3-term hi/lo error compensation:
    x @ w ~= xh@wh + xh@wl + xl@wh        (drop xl@wl, ~0.1% error)
X^T and W (pre-scaled by 32) are split hi/lo on the host and shipped fp8, so
the kernel needs no PE transposes for inputs. Scores are computed transposed
(S^T[k, q] with keys on partitions) so the attention weights can be consumed
directly as DoubleRow stationaries by attn @ V -- no attention transposes
either. exp() output is split hi/lo on device (ACT copy + DVE subtract).
The softmax denominator comes from an extra ones-column DoubleRow matmul and
cancels the 32x V scale exactly. The causal mask is one 128x128 f32 add per
key-chunk (two host mask tiles, selected by parity).
"""

import numpy as np

import concourse.bacc as bacc
import concourse.mybir as mybir
import concourse.tile as tile
from concourse import bass_utils

B, S, D = 4, 2048, 1024
P = 128
DCP = 4              # pairs of 128-deep contraction tiles (d dim)
ECP = 4              # pairs of 128-wide e tiles
NSLOT = 8            # q tiles per core
NQ = NSLOT * P       # 1024 q rows per core
NKC = S // P         # 16 key chunks
WSCALE = 32.0        # host pre-scale on all three weights
SCALE_EFF = 1.0 / (WSCALE * WSCALE * float(np.sqrt(np.float32(S))))
NEG = -1.0e9

F32 = mybir.dt.float32
FP8 = mybir.dt.float8e4
BF16 = mybir.dt.bfloat16
DR = mybir.MatmulPerfMode.DoubleRow


def build_attention_nc():
    nc = bacc.Bacc("TRN2", target_bir_lowering=False)

    xk_h = nc.dram_tensor("xk_h", [P, DCP, 2, S], FP8, kind="ExternalInput")
    xk_l = nc.dram_tensor("xk_l", [P, DCP, 2, S], FP8, kind="ExternalInput")
    xv_h = nc.dram_tensor("xv_h", [P, DCP, 2, S], FP8, kind="ExternalInput")
    xv_l = nc.dram_tensor("xv_l", [P, DCP, 2, S], FP8, kind="ExternalInput")
    xq_h = nc.dram_tensor("xq_h", [P, DCP, 2, NQ], FP8, kind="ExternalInput")
    xq_l = nc.dram_tensor("xq_l", [P, DCP, 2, NQ], FP8, kind="ExternalInput")
    w_in = {}
    for t in ("k", "v", "q"):
        comps = ("h", "l") if t == "v" else ("h",)
        for c in comps:
            w_in[t, c] = nc.dram_tensor(
                f"w{t}_{c}", [P, DCP, 2, D], FP8, kind="ExternalInput"
            )
    mask_a = nc.dram_tensor("mask_a", [P, P], F32, kind="ExternalInput")
    mask_b = nc.dram_tensor("mask_b", [P, P], F32, kind="ExternalInput")
    ones_in = nc.dram_tensor("ones32", [P, 2, 1], FP8, kind="ExternalInput")
    out = nc.dram_tensor("out", [NQ, D], BF16, kind="ExternalOutput")

    with tile.TileContext(nc) as tc:
        with (
            tc.tile_pool(name="res", bufs=1) as res,
            tc.tile_pool(name="wp", bufs=2) as wp,
            tc.tile_pool(name="xs", bufs=2) as xs,
            tc.tile_pool(name="tmpp", bufs=3) as tmpp,
            tc.tile_pool(name="outp", bufs=2) as outp,
            tc.tile_pool(name="recp", bufs=2) as recp,
        ):
            # scores run 1-term (hi only): kt/qt keep no lo component
            kt_h = res.tile([P, ECP, 2, S], FP8, tag="kth", name="kt_h")
            vv = {c: res.tile([P, NKC, D], FP8, tag=f"v{c}", name=f"v{c}") for c in "hl"}
            qt_h = res.tile([P, ECP, 2, NQ], FP8, tag="qth", name="qt_h")
            at = {c: res.tile([P, NKC, NQ], FP8, tag=f"at{c}", name=f"at{c}") for c in "hl"}
            ma_sb = res.tile([P, P], F32, tag="maska")
            mb_sb = res.tile([P, P], F32, tag="maskb")
            ones_sb = res.tile([P, 2, 1], FP8, tag="ones")

            def proj_psum(ps, stats, movs, stat_slice, mov_slice, terms):
                """len(terms) x 4 DR matmuls accumulating one psum group."""
                total = len(terms) * DCP
                n = 0
                for cs, cm in terms:
                    for dcp in range(DCP):
                        nc.tensor.matmul(
                            ps,
                            stat_slice(stats[cs], dcp),
                            mov_slice(movs[cm], dcp),
                            start=(n == 0),
                            stop=(n == total - 1),
                            perf_mode=DR,
                        )
                        n += 1

            def store_hilo(ps, hi_slice, lo_slice, hi_engine=None):
                eng = hi_engine or nc.scalar
                if eng is nc.scalar:
                    nc.scalar.copy(hi_slice, ps)
                else:
                    eng.tensor_copy(hi_slice, ps)
                nc.vector.tensor_tensor(
                    out=lo_slice, in0=ps, in1=hi_slice,
                    op=mybir.AluOpType.subtract,
                )

            # ================= K / V / Q projection =================
            with (
                tc.tile_pool(name="pp", bufs=6, space="PSUM") as pp,
                tc.tile_pool(name="pout", bufs=2, space="PSUM") as pout,
            ):
                wk = {"h": wp.tile([P, DCP, 2, D], FP8, tag="wh", name="wkh")}

                # ---- K: kt[e, keys] resident (2-term: x exact, w hi) ----
                for kb in range(4):
                    xk = {c: xs.tile([P, DCP, 2, 512], FP8, tag="xs", bufs=4, name=f"xk{c}") for c in "hl"}
                    if kb == 0:
                        # critical path: one FIFO queue, consumption order
                        nc.sync.dma_start(
                            wk["h"][:, 0:2, :, :], w_in["k", "h"][:, 0:2, :, :]
                        )
                        nc.sync.dma_start(
                            xk["h"][:, 0:2, :, :], xk_h[:, 0:2, :, 0:512]
                        )
                        nc.sync.dma_start(
                            wk["h"][:, 2:4, :, :], w_in["k", "h"][:, 2:4, :, :]
                        )
                        nc.sync.dma_start(
                            xk["h"][:, 2:4, :, :], xk_h[:, 2:4, :, 0:512]
                        )
                        nc.sync.dma_start(xk["l"], xk_l[:, :, :, 0:512])
                    else:
                        for c in "hl":
                            src = xk_h if c == "h" else xk_l
                            nc.sync.dma_start(
                                xk[c], src[:, :, :, kb * 512 : (kb + 1) * 512]
                            )
                    for ec in range(8):
                        ps = pp.tile([P, 512], F32, tag="ps")
                        proj_psum(
                            ps, wk, xk,
                            lambda w, dcp, ec=ec: w[:, dcp, :, ec * P : (ec + 1) * P],
                            lambda x, dcp: x[:, dcp, :, :],
                            terms=(("h", "h"), ("h", "l")),
                        )
                        nc.scalar.copy(
                            kt_h[:, ec // 2, ec % 2, kb * 512 : (kb + 1) * 512], ps
                        )

                # ---- V: v[keys, e] resident ----
                wv = {c: wp.tile([P, DCP, 2, D], FP8, tag=f"w{c}", name=f"wv{c}") for c in "hl"}
                # (V keeps full 3-term compensation)
                for c in "hl":
                    nc.sync.dma_start(wv[c], w_in["v", c][:, :, :, :])
                for kb in range(4):
                    xv = {c: xs.tile([P, DCP, 2, 512], FP8, tag="xs", bufs=4, name=f"xv{c}") for c in "hl"}
                    for c in "hl":
                        src = xv_h if c == "h" else xv_l
                        nc.sync.dma_start(xv[c], src[:, :, :, kb * 512 : (kb + 1) * 512])
                    for kti in range(4):
                        ktg = kb * 4 + kti
                        for eh in range(2):
                            ps = pp.tile([P, 512], F32, tag="ps")
                            proj_psum(
                                ps, xv, wv,
                                lambda x, dcp, kti=kti: x[:, dcp, :, kti * P : (kti + 1) * P],
                                lambda w, dcp, eh=eh: w[:, dcp, :, eh * 512 : (eh + 1) * 512],
                                terms=(("h", "h"), ("h", "l"), ("l", "h")),
                            )
                            store_hilo(
                                ps,
                                vv["h"][:, ktg, eh * 512 : (eh + 1) * 512],
                                vv["l"][:, ktg, eh * 512 : (eh + 1) * 512],
                            )

                # ---- Q: qt[e, q] resident ----
                wq = {"h": wp.tile([P, DCP, 2, D], FP8, tag="wh", name="wqh")}
                nc.sync.dma_start(wq["h"], w_in["q", "h"][:, :, :, :])
                xq = {c: xs.tile([P, DCP, 2, NQ], FP8, tag=f"xq{c}", bufs=1, name=f"xq{c}") for c in "hl"}
                nc.sync.dma_start(xq["h"], xq_h[:, :, :, :])
                nc.sync.dma_start(xq["l"], xq_l[:, :, :, :])
                for qb in range(2):
                    for ec in range(8):
                        ps = pp.tile([P, 512], F32, tag="ps")
                        proj_psum(
                            ps, wq, xq,
                            lambda w, dcp, ec=ec: w[:, dcp, :, ec * P : (ec + 1) * P],
                            lambda x, dcp, qb=qb: x[:, dcp, :, qb * 512 : (qb + 1) * 512],
                            terms=(("h", "h"), ("h", "l")),
                        )
                        nc.scalar.copy(
                            qt_h[:, ec // 2, ec % 2, qb * 512 : (qb + 1) * 512], ps
                        )

                # ============ attention (same PSUM pools stay open) ============

                def score_piece(t, pq0, wp_, masked):
                    """One S^T piece of k-chunk t: psum, mask, exp, a hi/lo."""
                    ps = pp.tile([P, 512], F32, tag="ps", name="ps")
                    for ecp in range(ECP):
                        nc.tensor.matmul(
                            ps[:, :wp_],
                            kt_h[:, ecp, :, t * P : (t + 1) * P],
                            qt_h[:, ecp, :, pq0 : pq0 + wp_],
                            start=(ecp == 0),
                            stop=(ecp == ECP - 1),
                            perf_mode=DR,
                        )
                    if masked:
                        nc.vector.tensor_tensor(
                            out=ps[:, 0:P], in0=ps[:, 0:P],
                            in1=(ma_sb if t % 2 == 0 else mb_sb),
                            op=mybir.AluOpType.add,
                        )
                    tmp = tmpp.tile([P, 512], F32, tag="tmp")
                    nc.scalar.activation(
                        out=tmp[:, :wp_], in_=ps[:, :wp_],
                        func=mybir.ActivationFunctionType.Exp,
                        scale=SCALE_EFF,
                    )
                    store_hilo(
                        tmp[:, :wp_],
                        at["h"][:, t, pq0 : pq0 + wp_],
                        at["l"][:, t, pq0 : pq0 + wp_],
                        hi_engine=nc.gpsimd,
                    )

                def score_pair(t0):
                    """Chunks (t0, t0+1): the narrow masked pieces (the ones
                    the next attn_v blocks on) first, then wides interleaved."""
                    q0 = P * (t0 // 2)
                    for t in (t0, t0 + 1):
                        score_piece(t, q0, P, masked=True)
                    wides = []
                    pq0 = q0 + P
                    while pq0 < NQ:
                        wp_ = min(512, NQ - pq0)
                        wides.append((pq0, wp_))
                        pq0 += wp_
                    for pq0, wp_ in wides:
                        for t in (t0, t0 + 1):
                            score_piece(t, pq0, wp_, masked=False)

                def attn_v_tail_group(s, ps_o, rec, out_sb, lo, hi, npair):
                    n = 0
                    w = hi - lo
                    for j in range(npair):
                        for ca, cv in (("h", "h"), ("h", "l"), ("l", "h")):
                            nc.tensor.matmul(
                                ps_o[:, 0 : w],
                                at[ca][:, 2 * j : 2 * j + 2, s * P : (s + 1) * P],
                                vv[cv][:, 2 * j : 2 * j + 2, lo:hi],
                                start=(n == 0),
                                stop=(n == 3 * npair - 1),
                                perf_mode=DR,
                            )
                            n += 1

                def attn_v(s):
                    """out[q-slot s, :] = (a @ 32V) / (a @ 32)."""
                    npair = s + 1
                    ps_den_t = pp.tile([P, 512], F32, tag="ps", name="ps_den")
                    ps_den = ps_den_t[:, 0:1]
                    n = 0
                    for j in range(npair):
                        for c in "hl":
                            nc.tensor.matmul(
                                ps_den,
                                at[c][:, 2 * j : 2 * j + 2, s * P : (s + 1) * P],
                                ones_sb,
                                start=(n == 0),
                                stop=(n == 2 * npair - 1),
                                perf_mode=DR,
                            )
                            n += 1
                    rec = recp.tile([P, 1], F32, tag="rec")
                    nc.vector.reciprocal(rec, ps_den)
                    out_sb = outp.tile([P, D], BF16, tag="outsb")
                    # the last slot splits eh1 into two 256-wide psum groups so
                    # its copy/DMA pipeline hides under real matmuls
                    groups = [(0, 512), (512, 1024)]
                    if s == NSLOT - 1:
                        groups = [(0, 512), (512, 768), (768, 1024)]
                    for gi, (lo, hi) in enumerate(groups):
                        ps_o = pout.tile([P, 512], F32, tag="po")
                        attn_v_tail_group(s, ps_o, rec, out_sb, lo, hi, npair)
                        eng = nc.scalar if gi % 2 == 0 else nc.vector
                        if eng is nc.scalar:
                            nc.scalar.activation(
                                out=out_sb[:, lo:hi],
                                in_=ps_o[:, 0 : hi - lo],
                                func=mybir.ActivationFunctionType.Copy,
                                scale=rec,
                            )
                        else:
                            nc.vector.tensor_scalar_mul(
                                out_sb[:, lo:hi], ps_o[:, 0 : hi - lo], rec
                            )
                        oq = nc.sync if gi % 2 == 0 else nc.scalar
                        oq.dma_start(
                            out[s * P : (s + 1) * P, lo:hi], out_sb[:, lo:hi]
                        )

                # pipeline: score chunks run one slot ahead of attn_v
                nc.scalar.dma_start(ma_sb, mask_a[:, :])
                nc.scalar.dma_start(mb_sb, mask_b[:, :])
                nc.scalar.dma_start(ones_sb, ones_in[:, :, :])
                score_pair(0)
                for s in range(NSLOT):
                    if s < NSLOT - 1:
                        score_pair(2 * s + 2)
                    attn_v(s)

    nc.compile()
    return nc


_NC_CACHE = None


def _get_nc():
    global _NC_CACHE
    if _NC_CACHE is None:
        _NC_CACHE = build_attention_nc()
    return _NC_CACHE


def _split8(a):
    import ml_dtypes

    f8 = ml_dtypes.float8_e4m3
    h = a.astype(np.float32).astype(f8)
    l = (a.astype(np.float32) - h.astype(np.float32)).astype(f8)
    return h, l


def _to_xt(a):
    """[rows, d] f32 component -> [128, DCP, 2, rows] fp8 (d on partitions)."""
    rows = a.shape[0]
    return np.ascontiguousarray(a.reshape(rows, DCP, 2, P).transpose(3, 1, 2, 0))


def _to_w(a):
    """[d, e] f32 component -> [128, DCP, 2, e] fp8 (d on partitions)."""
    return np.ascontiguousarray(a.reshape(DCP, 2, P, D).transpose(2, 0, 1, 3))


def _make_masks(h):
    i = np.arange(P)[:, None]
    j = np.arange(P)[None, :]
    if h == 0:
        a = np.where(j >= i, 0.0, NEG).astype(np.float32)  # k partition, q free
        b = np.full((P, P), NEG, dtype=np.float32)
    else:
        a = np.zeros((P, P), dtype=np.float32)
        b = np.where(j >= i, 0.0, NEG).astype(np.float32)
    return a, b


def kernel(
    inputs_for_keys,
    inputs_for_values,
    inputs_for_queries,
    weight_K,
    weight_V,
    weight_Q,
    trace=False,
):
    import ml_dtypes

    xk_full = np.asarray(inputs_for_keys, dtype=np.float32)
    xv_full = np.asarray(inputs_for_values, dtype=np.float32)
    xq_full = np.asarray(inputs_for_queries, dtype=np.float32)

    w_split = {}
    for name, w in (("k", weight_K), ("v", weight_V), ("q", weight_Q)):
        wh, wl = _split8(np.asarray(w, dtype=np.float32) * WSCALE)
        w_split[name] = (_to_w(wh), _to_w(wl))
    # K/Q projections run 2-term (w-hi only); only V ships its lo component

    xk_split = [tuple(_to_xt(c) for c in _split8(xk_full[b])) for b in range(B)]
    xv_split = [tuple(_to_xt(c) for c in _split8(xv_full[b])) for b in range(B)]

    ones32 = np.full((P, 2, 1), WSCALE, dtype=ml_dtypes.float8_e4m3)
    masks = [_make_masks(0), _make_masks(1)]

    in_maps = []
    for c in range(2 * B):
        b, h = c // 2, c % 2
        rows = np.concatenate(
            [
                xq_full[b, 256 * s + P * h : 256 * s + P * h + P, :]
                for s in range(NSLOT)
            ],
            axis=0,
        )
        qh, ql = _split8(rows)
        ma, mb = masks[h]
        in_maps.append(
            {
                "xk_h": xk_split[b][0], "xk_l": xk_split[b][1],
                "xv_h": xv_split[b][0], "xv_l": xv_split[b][1],
                "xq_h": _to_xt(qh), "xq_l": _to_xt(ql),
                "wk_h": w_split["k"][0],
                "wv_h": w_split["v"][0], "wv_l": w_split["v"][1],
                "wq_h": w_split["q"][0],
                "mask_a": ma, "mask_b": mb,
                "ones32": ones32,
            }
        )

    nc = _get_nc()
    res = bass_utils.run_bass_kernel_spmd(
        nc, in_maps, core_ids=list(range(2 * B)), trace=trace
    )

    out = np.empty((B, S, D), dtype=np.float32)
    for c in range(2 * B):
        b, h = c // 2, c % 2
        o = np.asarray(res.results[c]["out"], dtype=np.float32)
        for s in range(NSLOT):
            out[b, 256 * s + P * h : 256 * s + P * h + P, :] = o[
                s * P : (s + 1) * P, :
            ]

    if trace:
        return out, res
    return out


# revision 29
# speedup vs baseline: 2.1810x; 1.0317x over previous
"""Causal single-head attention (B=4, S=2048, D=1024) on 8 TRN2 NeuronCores.

Sharding: core c -> (batch b = c//2, half h = c%2). Every core runs the SAME
program: its 1024 query rows are 8 slots of 128 rows; slot s holds global
rows [256*s + 128*h, 256*s + 128*h + 128) of batch b, whose padded causal
key-length is 256*(s+1).

All matmuls run as fp8(e4m3) DoubleRow (2 contraction tiles per instruction,
0.5 cycles/row) with 3-term hi/lo error compensation:
    x @ w ~= xh@wh + xh@wl + xl@wh        (drop xl@wl, ~0.1% error)
X^T and W (pre-scaled by 32) are split hi/lo on the host and shipped fp8, so
the kernel needs no PE transposes for inputs. Scores are computed transposed
(S^T[k, q] with keys on partitions) so the attention weights can be consumed
directly as DoubleRow stationaries by attn @ V -- no attention transposes
either. exp() output is split hi/lo on device (ACT copy + DVE subtract).
The softmax denominator comes from an extra ones-column DoubleRow matmul and
cancels the 32x V scale exactly. The causal mask is one 128x128 f32 add per
key-chunk (two host mask tiles, selected by parity).
"""

import numpy as np

import concourse.bacc as bacc
import concourse.mybir as mybir
import concourse.tile as tile
from concourse import bass_utils

B, S, D = 4, 2048, 1024
P = 128
DCP = 4              # pairs of 128-deep contraction tiles (d dim)
ECP = 4              # pairs of 128-wide e tiles
NSLOT = 8            # q tiles per core
NQ = NSLOT * P       # 1024 q rows per core
NKC = S // P         # 16 key chunks
WSCALE = 32.0        # host pre-scale on all three weights
SCALE_EFF = 1.0 / (WSCALE * WSCALE * float(np.sqrt(np.float32(S))))
NEG = -1.0e9

F32 = mybir.dt.float32
FP8 = mybir.dt.float8e4
BF16 = mybir.dt.bfloat16
DR = mybir.MatmulPerfMode.DoubleRow


def build_attention_nc():
    nc = bacc.Bacc("TRN2", target_bir_lowering=False)

    xk_h = nc.dram_tensor("xk_h", [P, DCP, 2, S], FP8, kind="ExternalInput")
    xk_l = nc.dram_tensor("xk_l", [P, DCP, 2, S], FP8, kind="ExternalInput")
    xv_h = nc.dram_tensor("xv_h", [P, DCP, 2, S], FP8, kind="ExternalInput")
    xv_l = nc.dram_tensor("xv_l", [P, DCP, 2, S], FP8, kind="ExternalInput")
    xq_h = nc.dram_tensor("xq_h", [P, DCP, 2, NQ], FP8, kind="ExternalInput")
    xq_l = nc.dram_tensor("xq_l", [P, DCP, 2, NQ], FP8, kind="ExternalInput")
    w_in = {}
    for t in ("k", "v", "q"):
        comps = ("h", "l") if t == "v" else ("h",)
        for c in comps:
            w_in[t, c] = nc.dram_tensor(
                f"w{t}_{c}", [P, DCP, 2, D], FP8, kind="ExternalInput"
            )
    mask_a = nc.dram_tensor("mask_a", [P, P], F32, kind="ExternalInput")
    mask_b = nc.dram_tensor("mask_b", [P, P], F32, kind="ExternalInput")
    ones_in = nc.dram_tensor("ones32", [P, 2, 1], FP8, kind="ExternalInput")
    out = nc.dram_tensor("out", [NQ, D], BF16, kind="ExternalOutput")

    with tile.TileContext(nc) as tc:
        with (
            tc.tile_pool(name="res", bufs=1) as res,
            tc.tile_pool(name="wp", bufs=2) as wp,
            tc.tile_pool(name="xs", bufs=2) as xs,
            tc.tile_pool(name="tmpp", bufs=3) as tmpp,
            tc.tile_pool(name="outp", bufs=2) as outp,
            tc.tile_pool(name="recp", bufs=2) as recp,
        ):
            # scores run 1-term (hi only): kt/qt keep no lo component
            kt_h = res.tile([P, ECP, 2, S], FP8, tag="kth", name="kt_h")
            vv = {c: res.tile([P, NKC, D], FP8, tag=f"v{c}", name=f"v{c}") for c in "hl"}
            qt_h = res.tile([P, ECP, 2, NQ], FP8, tag="qth", name="qt_h")
            at = {c: res.tile([P, NKC, NQ], FP8, tag=f"at{c}", name=f"at{c}") for c in "hl"}
            ma_sb = res.tile([P, P], F32, tag="maska")
            mb_sb = res.tile([P, P], F32, tag="maskb")
            ones_sb = res.tile([P, 2, 1], FP8, tag="ones")

            def proj_psum(ps, stats, movs, stat_slice, mov_slice, terms):
                """len(terms) x 4 DR matmuls accumulating one psum group."""
                total = len(terms) * DCP
                n = 0
                for cs, cm in terms:
                    for dcp in range(DCP):
                        nc.tensor.matmul(
                            ps,
                            stat_slice(stats[cs], dcp),
                            mov_slice(movs[cm], dcp),
                            start=(n == 0),
                            stop=(n == total - 1),
                            perf_mode=DR,
                        )
                        n += 1

            def store_hilo(ps, hi_slice, lo_slice, hi_engine=None):
                eng = hi_engine or nc.scalar
                if eng is nc.scalar:
                    nc.scalar.copy(hi_slice, ps)
                else:
                    eng.tensor_copy(hi_slice, ps)
                nc.vector.tensor_tensor(
                    out=lo_slice, in0=ps, in1=hi_slice,
                    op=mybir.AluOpType.subtract,
                )

            # ================= K / V / Q projection =================
            with (
                tc.tile_pool(name="pp", bufs=6, space="PSUM") as pp,
                tc.tile_pool(name="pout", bufs=2, space="PSUM") as pout,
            ):
                wk = {"h": wp.tile([P, DCP, 2, D], FP8, tag="wh", name="wkh")}

                # ---- K: kt[e, keys] resident (2-term: x exact, w hi) ----
                for kb in range(4):
                    xk = {c: xs.tile([P, DCP, 2, 512], FP8, tag="xs", bufs=4, name=f"xk{c}") for c in "hl"}
                    if kb == 0:
                        # critical path: one FIFO queue, consumption order
                        nc.sync.dma_start(
                            wk["h"][:, 0:2, :, :], w_in["k", "h"][:, 0:2, :, :]
                        )
                        nc.sync.dma_start(
                            xk["h"][:, 0:2, :, :], xk_h[:, 0:2, :, 0:512]
                        )
                        nc.sync.dma_start(
                            wk["h"][:, 2:4, :, :], w_in["k", "h"][:, 2:4, :, :]
                        )
                        nc.sync.dma_start(
                            xk["h"][:, 2:4, :, :], xk_h[:, 2:4, :, 0:512]
                        )
                        nc.sync.dma_start(xk["l"], xk_l[:, :, :, 0:512])
                    else:
                        for c in "hl":
                            src = xk_h if c == "h" else xk_l
                            nc.sync.dma_start(
                                xk[c], src[:, :, :, kb * 512 : (kb + 1) * 512]
                            )
                    for ec in range(8):
                        ps = pp.tile([P, 512], F32, tag="ps")
                        proj_psum(
                            ps, wk, xk,
                            lambda w, dcp, ec=ec: w[:, dcp, :, ec * P : (ec + 1) * P],
                            lambda x, dcp: x[:, dcp, :, :],
                            terms=(("h", "h"), ("h", "l")),
                        )
                        nc.scalar.copy(
                            kt_h[:, ec // 2, ec % 2, kb * 512 : (kb + 1) * 512], ps
                        )

                # ---- Q: qt[e, q] resident ----
                wq = {"h": wp.tile([P, DCP, 2, D], FP8, tag="wh", name="wqh")}
                nc.sync.dma_start(wq["h"], w_in["q", "h"][:, :, :, :])
                xq = {c: xs.tile([P, DCP, 2, NQ], FP8, tag=f"xq{c}", bufs=1, name=f"xq{c}") for c in "hl"}
                nc.sync.dma_start(xq["h"], xq_h[:, :, :, :])
                nc.sync.dma_start(xq["l"], xq_l[:, :, :, :])
                for qb in range(2):
                    for ec in range(8):
                        ps = pp.tile([P, 512], F32, tag="ps")
                        proj_psum(
                            ps, wq, xq,
                            lambda w, dcp, ec=ec: w[:, dcp, :, ec * P : (ec + 1) * P],
                            lambda x, dcp, qb=qb: x[:, dcp, :, qb * 512 : (qb + 1) * 512],
                            terms=(("h", "h"), ("h", "l")),
                        )
                        nc.scalar.copy(
                            qt_h[:, ec // 2, ec % 2, qb * 512 : (qb + 1) * 512], ps
                        )

                # ---- V: v[keys, e] resident (emitted interleaved below) ----
                wv = {c: wp.tile([P, DCP, 2, D], FP8, tag=f"w{c}", name=f"wv{c}") for c in "hl"}
                # (V keeps full 3-term compensation)
                for c in "hl":
                    nc.sync.dma_start(wv[c], w_in["v", c][:, :, :, :])

                def v_block(kb):
                    xv = {c: xs.tile([P, DCP, 2, 512], FP8, tag="xs", bufs=4, name=f"xv{c}") for c in "hl"}
                    for c in "hl":
                        src = xv_h if c == "h" else xv_l
                        nc.sync.dma_start(xv[c], src[:, :, :, kb * 512 : (kb + 1) * 512])
                    for kti in range(4):
                        ktg = kb * 4 + kti
                        for eh in range(2):
                            ps = pp.tile([P, 512], F32, tag="ps")
                            proj_psum(
                                ps, xv, wv,
                                lambda x, dcp, kti=kti: x[:, dcp, :, kti * P : (kti + 1) * P],
                                lambda w, dcp, eh=eh: w[:, dcp, :, eh * 512 : (eh + 1) * 512],
                                terms=(("h", "h"), ("h", "l"), ("l", "h")),
                            )
                            store_hilo(
                                ps,
                                vv["h"][:, ktg, eh * 512 : (eh + 1) * 512],
                                vv["l"][:, ktg, eh * 512 : (eh + 1) * 512],
                            )

                # ============ attention (same PSUM pools stay open) ============

                def score_piece(t, pq0, wp_, masked):
                    """One S^T piece of k-chunk t: psum, mask, exp, a hi/lo."""
                    ps = pp.tile([P, 512], F32, tag="ps", name="ps")
                    for ecp in range(ECP):
                        nc.tensor.matmul(
                            ps[:, :wp_],
                            kt_h[:, ecp, :, t * P : (t + 1) * P],
                            qt_h[:, ecp, :, pq0 : pq0 + wp_],
                            start=(ecp == 0),
                            stop=(ecp == ECP - 1),
                            perf_mode=DR,
                        )
                    if masked:
                        nc.vector.tensor_tensor(
                            out=ps[:, 0:P], in0=ps[:, 0:P],
                            in1=(ma_sb if t % 2 == 0 else mb_sb),
                            op=mybir.AluOpType.add,
                        )
                    tmp = tmpp.tile([P, 512], F32, tag="tmp")
                    nc.scalar.activation(
                        out=tmp[:, :wp_], in_=ps[:, :wp_],
                        func=mybir.ActivationFunctionType.Exp,
                        scale=SCALE_EFF,
                    )
                    store_hilo(
                        tmp[:, :wp_],
                        at["h"][:, t, pq0 : pq0 + wp_],
                        at["l"][:, t, pq0 : pq0 + wp_],
                        hi_engine=nc.gpsimd,
                    )

                def score_pair(t0):
                    """Chunks (t0, t0+1): the narrow masked pieces (the ones
                    the next attn_v blocks on) first, then wides interleaved."""
                    q0 = P * (t0 // 2)
                    for t in (t0, t0 + 1):
                        score_piece(t, q0, P, masked=True)
                    wides = []
                    pq0 = q0 + P
                    while pq0 < NQ:
                        wp_ = min(512, NQ - pq0)
                        wides.append((pq0, wp_))
                        pq0 += wp_
                    for pq0, wp_ in wides:
                        for t in (t0, t0 + 1):
                            score_piece(t, pq0, wp_, masked=False)

                def attn_v_tail_group(s, ps_o, rec, out_sb, lo, hi, npair):
                    n = 0
                    w = hi - lo
                    for j in range(npair):
                        for ca, cv in (("h", "h"), ("h", "l"), ("l", "h")):
                            nc.tensor.matmul(
                                ps_o[:, 0 : w],
                                at[ca][:, 2 * j : 2 * j + 2, s * P : (s + 1) * P],
                                vv[cv][:, 2 * j : 2 * j + 2, lo:hi],
                                start=(n == 0),
                                stop=(n == 3 * npair - 1),
                                perf_mode=DR,
                            )
                            n += 1

                def attn_v(s):
                    """out[q-slot s, :] = (a @ 32V) / (a @ 32)."""
                    npair = s + 1
                    ps_den_t = pp.tile([P, 512], F32, tag="ps", name="ps_den")
                    ps_den = ps_den_t[:, 0:1]
                    n = 0
                    for j in range(npair):
                        for c in "hl":
                            nc.tensor.matmul(
                                ps_den,
                                at[c][:, 2 * j : 2 * j + 2, s * P : (s + 1) * P],
                                ones_sb,
                                start=(n == 0),
                                stop=(n == 2 * npair - 1),
                                perf_mode=DR,
                            )
                            n += 1
                    rec = recp.tile([P, 1], F32, tag="rec")
                    nc.vector.reciprocal(rec, ps_den)
                    out_sb = outp.tile([P, D], BF16, tag="outsb")
                    # the last slot splits eh1 into two 256-wide psum groups so
                    # its copy/DMA pipeline hides under real matmuls
                    groups = [(0, 512), (512, 1024)]
                    if s == NSLOT - 1:
                        groups = [(0, 512), (512, 768), (768, 1024)]
                    for gi, (lo, hi) in enumerate(groups):
                        ps_o = pout.tile([P, 512], F32, tag="po")
                        attn_v_tail_group(s, ps_o, rec, out_sb, lo, hi, npair)
                        eng = nc.scalar if gi % 2 == 0 else nc.vector
                        if eng is nc.scalar:
                            nc.scalar.activation(
                                out=out_sb[:, lo:hi],
                                in_=ps_o[:, 0 : hi - lo],
                                func=mybir.ActivationFunctionType.Copy,
                                scale=rec,
                            )
                        else:
                            nc.vector.tensor_scalar_mul(
                                out_sb[:, lo:hi], ps_o[:, 0 : hi - lo], rec
                            )
                        oq = nc.sync if gi % 2 == 0 else nc.scalar
                        oq.dma_start(
                            out[s * P : (s + 1) * P, lo:hi], out_sb[:, lo:hi]
                        )

                # pipeline: score chunks run one slot ahead of attn_v
                nc.scalar.dma_start(ma_sb, mask_a[:, :])
                nc.scalar.dma_start(mb_sb, mask_b[:, :])
                nc.scalar.dma_start(ones_sb, ones_in[:, :, :])
                # V blocks interleaved with attention: V's PE work hides
                # the exp pipeline latency of freshly produced score chunks
                v_block(0)
                score_pair(0)
                v_block(1)
                score_pair(2)
                attn_v(0)
                score_pair(4)
                v_block(2)
                attn_v(1)
                score_pair(6)
                attn_v(2)
                score_pair(8)
                v_block(3)
                attn_v(3)
                score_pair(10)
                attn_v(4)
                score_pair(12)
                attn_v(5)
                score_pair(14)
                attn_v(6)
                attn_v(7)

    nc.compile()
    return nc


_NC_CACHE = None


def _get_nc():
    global _NC_CACHE
    if _NC_CACHE is None:
        _NC_CACHE = build_attention_nc()
    return _NC_CACHE


def _split8(a):
    import ml_dtypes

    f8 = ml_dtypes.float8_e4m3
    h = a.astype(np.float32).astype(f8)
    l = (a.astype(np.float32) - h.astype(np.float32)).astype(f8)
    return h, l


def _to_xt(a):
    """[rows, d] f32 component -> [128, DCP, 2, rows] fp8 (d on partitions)."""
    rows = a.shape[0]
    return np.ascontiguousarray(a.reshape(rows, DCP, 2, P).transpose(3, 1, 2, 0))


def _to_w(a):
    """[d, e] f32 component -> [128, DCP, 2, e] fp8 (d on partitions)."""
    return np.ascontiguousarray(a.reshape(DCP, 2, P, D).transpose(2, 0, 1, 3))


def _make_masks(h):
    i = np.arange(P)[:, None]
    j = np.arange(P)[None, :]
    if h == 0:
        a = np.where(j >= i, 0.0, NEG).astype(np.float32)  # k partition, q free
        b = np.full((P, P), NEG, dtype=np.float32)
    else:
        a = np.zeros((P, P), dtype=np.float32)
        b = np.where(j >= i, 0.0, NEG).astype(np.float32)
    return a, b


def kernel(
    inputs_for_keys,
    inputs_for_values,
    inputs_for_queries,
    weight_K,
    weight_V,
    weight_Q,
    trace=False,
):
    import ml_dtypes

    xk_full = np.asarray(inputs_for_keys, dtype=np.float32)
    xv_full = np.asarray(inputs_for_values, dtype=np.float32)
    xq_full = np.asarray(inputs_for_queries, dtype=np.float32)

    w_split = {}
    for name, w in (("k", weight_K), ("v", weight_V), ("q", weight_Q)):
        wh, wl = _split8(np.asarray(w, dtype=np.float32) * WSCALE)
        w_split[name] = (_to_w(wh), _to_w(wl))
    # K/Q projections run 2-term (w-hi only); only V ships its lo component

    xk_split = [tuple(_to_xt(c) for c in _split8(xk_full[b])) for b in range(B)]
    xv_split = [tuple(_to_xt(c) for c in _split8(xv_full[b])) for b in range(B)]

    ones32 = np.full((P, 2, 1), WSCALE, dtype=ml_dtypes.float8_e4m3)
    masks = [_make_masks(0), _make_masks(1)]

    in_maps = []
    for c in range(2 * B):
        b, h = c // 2, c % 2
        rows = np.concatenate(
            [
                xq_full[b, 256 * s + P * h : 256 * s + P * h + P, :]
                for s in range(NSLOT)
            ],
            axis=0,
        )
        qh, ql = _split8(rows)
        ma, mb = masks[h]
        in_maps.append(
            {
                "xk_h": xk_split[b][0], "xk_l": xk_split[b][1],
                "xv_h": xv_split[b][0], "xv_l": xv_split[b][1],
                "xq_h": _to_xt(qh), "xq_l": _to_xt(ql),
                "wk_h": w_split["k"][0],
                "wv_h": w_split["v"][0], "wv_l": w_split["v"][1],
                "wq_h": w_split["q"][0],
                "mask_a": ma, "mask_b": mb,
                "ones32": ones32,
            }
        )

    nc = _get_nc()
    res = bass_utils.run_bass_kernel_spmd(
        nc, in_maps, core_ids=list(range(2 * B)), trace=trace
    )

    out = np.empty((B, S, D), dtype=np.float32)
    for c in range(2 * B):
        b, h = c // 2, c % 2
        o = np.asarray(res.results[c]["out"], dtype=np.float32)
        for s in range(NSLOT):
            out[b, 256 * s + P * h : 256 * s + P * h + P, :] = o[
                s * P : (s + 1) * P, :
            ]

    if trace:
        return out, res
    return out
